# revision 45
# baseline (speedup 1.0000x reference)
"""Trainium2 Bass kernel: GQA attention block (B=1, S=2048, DIM=4096, 32 Q / 8 KV
heads, HD=128, RoPE, causal mask, o_proj), tensor-parallel over 8 NeuronCores.

Sharding (per core c):
  - Q heads 4c..4c+3 (wq rows 512c..512c+512), KV head c (wk/wv rows 128c..).
  - x replicated; each core computes qkv projections + RoPE + causal attention
    for its heads, producing ctx^T [512 local features, 2048 seq] in bf16.
  - AllGather over the feature axis -> ctx^T full [4096, 2048], then each core
    computes o_proj for its 512 output columns (wo rows 512c..512c+512).
  - Host concatenates the per-core output column blocks.

Schedule notes (trace-driven):
  - PE is the bottleneck (~1630 512-free matmuls) and the board GPIO power
    throttle caps the sustained clock at 13/16 x 2.4GHz; idle gaps >3.4us
    additionally re-throttle to half (HAM). Goal: a gapless PE stream.
  - Phase order: [proj_k + attn_k for k=0..3] back-to-back, then ALL four
    o_proj phases at the end. Every AllGather (one 512KB op per sb, ~20-35us,
    serial on one CC stream) lands with ~100us of slack before its consumer,
    which also absorbs the 30-180us run-to-run CC stream-init stagger.
  - gpsimd carries ONLY the collective triggers (+ startup consts): anything
    else head-of-line-blocks the AG trigger behind data waits. ctx/rope-swap
    DMAs ride the two HWDGE queues; the next sb's x DMAs are pre-emitted at
    pass-B so they sit ahead of attention's DMAs in FIFO order.
  - sb0 projections run one 6-accumulator pass (q0..q3+k+v, 2 PSUM banks
    borrowed from the idle score pool) to halve startup DMA demand per cycle.
  - Softmax normalize stays on-chip: bf16 denominator matmul, DVE reciprocal,
    rank-1 PE matmul broadcasts 1/den across partitions (no DRAM round trip);
    softmax SCALE is folded into the Exp activation so q/k share rope tables.
  - Diagonal score/PV matmuls shrink their free (query) range causally.
"""

import numpy as np
import ml_dtypes

B, S, DIM = 1, 2048, 4096
H, HKV, HD = 32, 8, 128
N_CORES = 8
QH = H // N_CORES            # 4 local q heads
OC = QH * HD                 # 512 local q/out columns
SB = 512                     # seq block
NSB = S // SB                # 4
KT = DIM // 128              # 32 contraction tiles
SCALE = HD ** -0.5
NEG = -1e9

bf16 = ml_dtypes.bfloat16

_CACHE = {}


def _build_nc():
    import contextlib
    import concourse.tile as tile
    from concourse import bacc, bass_isa, library_config, mybir

    f32 = mybir.dt.float32
    bft = mybir.dt.bfloat16
    AF = mybir.ActivationFunctionType
    ALU = mybir.AluOpType

    nc = bacc.Bacc("TRN2")

    # pre-tiled on host: xt4[sb][c4] -> [128, 4, SB] contiguous; wq8[j] ->
    # [128, 4, OC] contiguous (k-tiles 4j..4j+3); wkv -> [128, KT, HD]
    xt_p = nc.declare_dram_parameter("xt_p", [NSB, KT // 4, 128, 4, SB], bft, isOutput=False)
    wq8 = nc.declare_dram_parameter("wq8", [8, 128, KT // 8, OC], bft, isOutput=False)
    wk_p = nc.declare_dram_parameter("wk_p", [128, KT, HD], bft, isOutput=False)
    wv_p = nc.declare_dram_parameter("wv_p", [128, KT, HD], bft, isOutput=False)
    wo4 = nc.declare_dram_parameter("wo4", [4, 128, KT // 4, OC], bft, isOutput=False)
    bqc = nc.declare_dram_parameter("bqc", [128, QH], mybir.dt.float32, isOutput=False)
    bkc = nc.declare_dram_parameter("bkc", [128, 1], mybir.dt.float32, isOutput=False)
    bvc = nc.declare_dram_parameter("bvc", [128, 1], mybir.dt.float32, isOutput=False)
    tkc = nc.declare_dram_parameter("tkc", [128, S], bft, isOutput=False)
    tks = nc.declare_dram_parameter("tks", [128, S], bft, isOutput=False)
    cmask = nc.declare_dram_parameter("cmask", [4, 128, SB], bft, isOutput=False)
    ident = nc.declare_dram_parameter("ident", [128, 128], bft, isOutput=False)
    outT = nc.declare_dram_parameter("outT", [OC, S], bft, isOutput=True)

    cc_warm_in = nc.dram_tensor("cc_warm_in", [1, 128], mybir.dt.float32)
    cc_warm_out = nc.dram_tensor("cc_warm_out", [N_CORES, 128], mybir.dt.float32,
                                 addr_space="Shared")
    cc_in = [nc.dram_tensor(f"cc_in{sb}", [OC, SB], bft) for sb in range(NSB)]
    # one AllGather per sb: all o_proj PE work is stacked at the END of the
    # kernel (after sb3's attention), so every gather lands with ~100us of
    # slack before its consumer -- fewer ops under the ~15-20us/op fixed cost,
    # and robust to the run-to-run CC stream-init stagger.
    cc_out = [nc.dram_tensor(f"cc_out{sb}", [N_CORES * OC, SB], bft,
                             addr_space="Shared") for sb in range(NSB)]

    with tile.TileContext(nc) as tc:
        with contextlib.ExitStack() as ctx:
            consts = ctx.enter_context(tc.tile_pool(name="consts", bufs=1))
            # 12 x-slots: half of each next-sb's loads get instantly-free ring
            # slots, so the x stream runs ahead of the AllGather windows
            # (whose DMA traffic throttles concurrent queue throughput)
            xpool = ctx.enter_context(tc.tile_pool(name="xpool", bufs=12))
            persist = ctx.enter_context(tc.tile_pool(name="persist", bufs=4))
            qpool = ctx.enter_context(tc.tile_pool(name="qpool", bufs=2))
            rtmp = ctx.enter_context(tc.tile_pool(name="rtmp", bufs=2))
            ppool = ctx.enter_context(tc.tile_pool(name="ppool", bufs=6))
            npool = ctx.enter_context(tc.tile_pool(name="npool", bufs=2))
            dsum = ctx.enter_context(tc.tile_pool(name="dsum", bufs=2))
            cpool = ctx.enter_context(tc.tile_pool(name="cpool", bufs=2))
            opool = ctx.enter_context(tc.tile_pool(name="opool", bufs=3))

            ps_acc = ctx.enter_context(tc.tile_pool(name="ps_acc", bufs=4, space="PSUM"))
            ps_sc = ctx.enter_context(tc.tile_pool(name="ps_sc", bufs=3, space="PSUM"))
            ps_cx = ctx.enter_context(tc.tile_pool(name="ps_cx", bufs=1, space="PSUM"))

            # tiny dummy collective, FIRST gpsimd instruction: queues behind
            # the runtime's CC init barrier so the first real AllGather starts
            # without the ~45us stream-init latency. The gathered bytes are
            # garbage and never read.
            nc.gpsimd.collective_compute(
                "AllGather", ALU.bypass,
                replica_groups=[list(range(N_CORES))],
                ins=[cc_warm_in[:].opt()],
                outs=[cc_warm_out[:].opt()],
            )
            # partition_all_reduce (softmax denominator) lives in the attn
            # gpsimd library
            nc.gpsimd.load_library(library_config.attn)

            # ---- priority DMAs: exactly what the first projection MMs need,
            # spread over FOUR trigger queues so no single queue serializes the
            # startup: sync=x stream, scalar=wq chunks, vector=wk/wv pieces
            # (+ident, then wo), gpsimd=rope tables+mask.
            KC = KT // 4
            KQ = KT // 8
            wq_ch = [None] * 8

            def load_wq_chunk(j, eng=None):
                if j < 8 and wq_ch[j] is None:
                    w = consts.tile([128, KQ, OC], bft, tag=f"wq{j}", name=f"wq{j}")
                    (eng or nc.scalar).dma_start(out=w[:], in_=wq8[j])
                    wq_ch[j] = w

            # memset for PE warmup first on the vector FIFO (no deps, fast)
            wtile = consts.tile([128, SB], bft)
            nc.vector.memset(wtile[:], 0.0)

            xt_first = xpool.tile([128, 4, SB], bft, tag="xt", name="xt")
            nc.sync.dma_start(out=xt_first[:], in_=xt_p[0, 0])
            wk_sb = consts.tile([128, KT, HD], bft)
            wv_sb = consts.tile([128, KT, HD], bft)

            def load_kv_piece(lo, hi, eng):
                eng.dma_start(out=wk_sb[:, lo:hi, :], in_=wk_p[:, lo:hi, :])
                eng.dma_start(out=wv_sb[:, lo:hi, :], in_=wv_p[:, lo:hi, :])

            # scalar = pure wq stream (4MB, chunk j lands well before its
            # k-tiles are reached); sync = x + wk/wv interleaved in
            # consumption order. ~5MB each by pass-A end.
            load_wq_chunk(0)
            load_wq_chunk(1)
            bqc_sb = consts.tile([128, QH], f32)
            nc.sync.dma_start(out=bqc_sb[:], in_=bqc[:])
            bkc_sb = consts.tile([128, 1], f32)
            nc.sync.dma_start(out=bkc_sb[:], in_=bkc[:])
            bvc_sb = consts.tile([128, 1], f32)
            nc.sync.dma_start(out=bvc_sb[:], in_=bvc[:])
            ident_sb = consts.tile([128, 128], bft)
            nc.sync.dma_start(out=ident_sb[:], in_=ident[:])
            load_kv_piece(0, 4, nc.sync)
            load_kv_piece(4, 8, nc.sync)
            # remaining wk/wv pieces interleave with sb0's x stream on sync
            # (emitted inside the c4 loop, keyed by consumption time)
            kv_pieces = {c4: [(4 * c4 + 4, 4 * c4 + 8)] for c4 in range(1, 7)}

            # PE warmup: keep the clock up while the first DMAs land. Short --
            # the first projection matmuls should be ready right after.
            warm_ps = ps_sc.tile([128, SB], f32, tag="sc_ps", name="sc_ps")
            for i in range(10):
                nc.tensor.matmul(warm_ps[:], wtile[:, 0:128], wtile[:],
                                 start=(i == 0), stop=(i == 9))

            # bulk consts on gpsimd: rope tables + mask (needed ~45us). One
            # cos/sin table pair serves q AND k -- the softmax SCALE is folded
            # into the Exp activation's scale operand instead of q's tables.
            tkc_sb = consts.tile([128, S], bft)
            nc.gpsimd.dma_start(out=tkc_sb[:], in_=tkc[:])
            tks_sb = consts.tile([128, S], bft)
            nc.gpsimd.dma_start(out=tks_sb[:], in_=tks[:])
            cm_sb = consts.tile([128, 4, SB], bft)
            nc.gpsimd.dma_start(out=cm_sb[:], in_=cmask.rearrange("j p q -> p j q"))
            # o_proj weight tiles: DMAs emitted on scalar at sb0 pass B (behind
            # the projection weights, ahead of sb1's x stream; needed ~150us)
            wo_ch = [consts.tile([128, KC, OC], bft, tag=f"wo{j}", name=f"wo{j}")
                     for j in range(4)]


            def wq_at(kt, lo, hi):
                return wq_ch[kt // KQ][:, kt % KQ, lo:hi]

            def wo_at(kt, lo, hi):
                return wo_ch[kt // KC][:, kt % KC, lo:hi]

            # persistent per-sb K^T and V tiles
            k_tiles = [None] * NSB   # [128 d, SB s] bf16
            v_tiles = [None] * NSB   # [128 s, 4, 128 d] bf16

            rope_ctr = [0]

            def rope(dst, src, tcos, tsin, s0):
                # dst/src: [128, SB] bf16. tcos/tsin have cos/sin duplicated in
                # both partition halves. swp = src with halves swapped (DMA
                # partition move), so every DVE op is partition-aligned.
                # Swap DMAs ride the HWDGE queues (alternating): gpsimd must
                # stay empty so AllGather triggers fire the moment ctx lands
                # (the tile scheduler queues swp DMAs ahead of AG triggers,
                # and a swp head-of-line-blocks until its qraw exists).
                cL = tcos[0:64, s0:s0 + SB]
                cH = tcos[64:128, s0:s0 + SB]
                sL = tsin[0:64, s0:s0 + SB]
                sH = tsin[64:128, s0:s0 + SB]
                swp = rtmp.tile([128, SB], bft, tag="ropeswp", name="ropeswp")
                seng = nc.sync if rope_ctr[0] % 2 == 0 else nc.scalar
                rope_ctr[0] += 1
                seng.dma_start(out=swp[0:64, :], in_=src[64:128, :])
                seng.dma_start(out=swp[64:128, :], in_=src[0:64, :])
                tA = rtmp.tile([128, SB], bft, tag="ropetA", name="ropetA")
                tB = rtmp.tile([128, SB], bft, tag="ropetB", name="ropetB")
                nc.vector.tensor_tensor(tA[0:64, :], src[0:64, :], cL, ALU.mult)
                nc.vector.tensor_tensor(tA[64:128, :], swp[64:128, :], sH, ALU.mult)
                nc.vector.tensor_tensor(tB[0:64, :], swp[0:64, :], sL, ALU.mult)
                nc.vector.tensor_tensor(tB[64:128, :], src[64:128, :], cH, ALU.mult)
                nc.vector.tensor_tensor(dst[0:64, :], tA[0:64, :], tB[0:64, :], ALU.subtract)
                nc.vector.tensor_tensor(dst[64:128, :], tA[64:128, :], tB[64:128, :], ALU.add)

            def o_proj(sb):
                s0 = sb * SB
                o_ps = [ps_acc.tile([128, SB], f32, tag="acc", name="acc") for _ in range(QH)]
                # cc_out rows: core*512 + head*128 + p, so chunk feature-tile
                # b = c4*4+k4 maps 1:1 to the wo contraction tile index
                co_r = cc_out[sb].rearrange("(c b p) s -> c p b s", p=128, b=4)
                for c4 in range(8):
                    rt4 = opool.tile([128, 4, SB], bft, tag="rt", name="rt", bufs=3)
                    eng = nc.sync if c4 % 2 == 0 else nc.scalar
                    eng.dma_start(out=rt4[:], in_=co_r[c4])
                    for k4 in range(4):
                        ft = c4 * 4 + k4
                        for ct in range(QH):
                            nc.tensor.matmul(
                                o_ps[ct][:], wo_at(ft, ct * 128, (ct + 1) * 128),
                                rt4[:, k4, :],
                                start=(ft == 0), stop=(ft == KT - 1),
                            )
                for ct in range(QH):
                    ot = opool.tile([128, SB], bft, tag="ot", name="ot")
                    nc.vector.tensor_copy(ot[:], o_ps[ct][:])
                    nc.scalar.dma_start(
                        out=outT[ct * 128:(ct + 1) * 128, s0:s0 + SB], in_=ot[:]
                    )

            xt_next = [None] * (KT // 4)
            for sb in range(NSB):
                s0 = sb * SB
                q_sb = [None] * QH

                # ---- projection pass A ----
                # sb0 runs a SINGLE 6-accumulator pass (q0..q3 + k + v): two
                # extra accumulators borrowed from the idle score pool (no
                # attention overlaps sb0's projections). 6 MMs per k-tile
                # halves the startup DMA demand per PE-cycle, which is what
                # the two HWDGE queues can actually sustain while weights and
                # x stream in cold.
                na = QH if sb == 0 else 2
                qa_ps = [ps_acc.tile([128, SB], f32, tag="acc", name="acc") for _ in range(2)]
                if sb == 0:
                    qa_ps += [ps_sc.tile([128, SB], f32, tag="sc_ps", name="sc_ps")
                              for _ in range(2)]
                k_ps = ps_acc.tile([128, SB], f32, tag="acc", name="acc")
                v_ps = ps_acc.tile([128, SB], f32, tag="acc", name="acc")
                xt_chunks = [None] * (KT // 4)
                for c4 in range(KT // 4):
                    if sb == 0:
                        if c4 == 0:
                            xt4 = xt_first
                        else:
                            # sb0's x all on sync: scalar stays a pure wq
                            # stream so neither starves the cold ramp
                            xt4 = xpool.tile([128, 4, SB], bft, tag="xt", name="xt")
                            nc.sync.dma_start(out=xt4[:], in_=xt_p[sb, c4])
                        load_wq_chunk(c4 + 2)
                        for lo, hi in kv_pieces.get(c4, []):
                            load_kv_piece(lo, hi, nc.sync)
                    else:
                        # pre-emitted at the previous sb's pass-B start
                        xt4 = xt_next[c4]
                    xt_chunks[c4] = xt4
                    for k4 in range(4):
                        kt = c4 * 4 + k4
                        xt = xt4[:, k4, :]
                        st = (kt == 0)
                        sp = (kt == KT - 1)
                        for h in range(na):
                            nc.tensor.matmul(
                                qa_ps[h][:], wq_at(kt, h * 128, (h + 1) * 128), xt,
                                start=st, stop=sp,
                            )
                        nc.tensor.matmul(k_ps[:], wk_sb[:, kt, :], xt, start=st, stop=sp)
                        nc.tensor.matmul(v_ps[:], wv_sb[:, kt, :], xt, start=st, stop=sp)

                if sb == 0:
                    # sb0's chunks are fully read now: pre-emit sb1's x DMAs
                    # (slots recycle progressively), then wo on scalar (which
                    # is idle after the wq stream; needed only at ~300us)
                    for c4 in range(KT // 4):
                        t = xpool.tile([128, 4, SB], bft, tag="xt", name="xt")
                        eng = nc.sync if c4 % 2 == 0 else nc.scalar
                        eng.dma_start(out=t[:], in_=xt_p[1, c4])
                        xt_next[c4] = t
                    for j in range(4):
                        nc.scalar.dma_start(out=wo_ch[j][:], in_=wo4[j])

                for h in range(na):
                    qraw = qpool.tile([128, SB], bft, tag="qraw", name="qraw")
                    nc.scalar.activation(qraw[:], qa_ps[h][:], AF.Identity,
                                         bias=bqc_sb[:, h:h + 1])
                    qr = qpool.tile([128, SB], bft, tag="qrope", name="qrope", bufs=6)
                    rope(qr, qraw, tkc_sb, tks_sb, s0)
                    q_sb[h] = qr
                kraw = qpool.tile([128, SB], bft, tag="kraw", name="kraw")
                nc.scalar.activation(kraw[:], k_ps[:], AF.Identity,
                                     bias=bkc_sb[:, 0:1])
                k_t = persist.tile([128, SB], bft, tag="k_t", name="k_t")
                rope(k_t, kraw, tkc_sb, tks_sb, s0)
                k_tiles[sb] = k_t
                vraw = qpool.tile([128, SB], bft, tag="vraw", name="vraw")
                nc.scalar.activation(vraw[:], v_ps[:], AF.Identity,
                                     bias=bvc_sb[:, 0:1])
                # transpose V on the PE (vraw_tile.T @ I) -- DMA_TRANSPOSE
                # serializes behind in-flight collectives, this doesn't
                vt_ps = ps_sc.tile([128, QH, 128], f32, tag="sc_ps", name="sc_ps")
                for i in range(QH):
                    nc.tensor.matmul(vt_ps[:, i, :], vraw[:, i * 128:(i + 1) * 128],
                                     ident_sb[:], start=True, stop=True)
                v_t = persist.tile([128, QH, 128], bft, tag="v_t", name="v_t")
                nc.vector.tensor_copy(v_t[:], vt_ps[:])
                v_tiles[sb] = v_t

                # ---- projection pass B (sb>0): q2, q3, x chunks resident.
                # Forward c4 order so chunk slots free progressively.
                if sb > 0:
                    qb_ps = [ps_acc.tile([128, SB], f32, tag="acc", name="acc") for _ in range(2)]
                    first_b = True
                    for c4 in range(KT // 4):
                        xt4 = xt_chunks[c4]
                        for k4 in range(4):
                            kt = c4 * 4 + k4
                            xt = xt4[:, k4, :]
                            last_b = (c4 == KT // 4 - 1 and k4 == 3)
                            for h in range(2):
                                nc.tensor.matmul(
                                    qb_ps[h][:], wq_at(kt, (2 + h) * 128, (3 + h) * 128), xt,
                                    start=first_b, stop=last_b,
                                )
                            first_b = False
                    # pre-emit the NEXT sb's x DMAs (after pass B's reads so
                    # the WAR slot reuse is tracked; the DMAs still fire
                    # during pass B execution, ahead of this sb's ctx DMAs)
                    if sb + 1 < NSB:
                        for c4 in range(KT // 4):
                            t = xpool.tile([128, 4, SB], bft, tag="xt", name="xt")
                            eng = nc.sync if c4 % 2 == 0 else nc.scalar
                            eng.dma_start(out=t[:], in_=xt_p[sb + 1, c4])
                            xt_next[c4] = t
                    for h in range(2):
                        qraw = qpool.tile([128, SB], bft, tag="qraw", name="qraw")
                        nc.scalar.activation(qraw[:], qb_ps[h][:], AF.Identity,
                                             bias=bqc_sb[:, 2 + h:3 + h])
                        qr = qpool.tile([128, SB], bft, tag="qrope", name="qrope", bufs=6)
                        rope(qr, qraw, tkc_sb, tks_sb, s0)
                        q_sb[2 + h] = qr

                # ---- causal attention for q-block sb, 4 heads ----
                nkt2 = 4 * (sb + 1)
                for h in range(QH):
                    # sb3 (the longest attention block, no projection filler
                    # after it): every head gets its OWN ctx bank from ps_acc
                    # (free: no proj interleaves behind attn3), removing the
                    # per-head serialization through the single ps_cx bank.
                    if sb == 3:
                        ctx_ps = ps_acc.tile([128, SB], f32, tag="acc", name="acc")
                    else:
                        ctx_ps = ps_cx.tile([128, SB], f32, tag="ctx_ps", name="ctx_ps")

                    ds = dsum.tile([128, SB], f32, tag="ds", name="ds")

                    def emit_score(kt2):
                        ksb, ki = divmod(kt2, 4)
                        off = 128 * ki if ksb == sb else 0
                        sc_ps = ps_sc.tile([128, SB], f32, tag="sc_ps", name="sc_ps")
                        nc.tensor.matmul(
                            sc_ps[:, off:],
                            k_tiles[ksb][:, ki * 128:(ki + 1) * 128],
                            q_sb[h][:, off:],
                            start=True, stop=True,
                        )
                        probs = ppool.tile([128, SB], bft, tag="probs", name="probs", bufs=6)
                        nc.scalar.activation(probs[:, off:], sc_ps[:, off:], AF.Exp,
                                             scale=float(SCALE))
                        if ksb == sb:
                            psel = ppool.tile([128, SB], bft, tag="psel", name="psel", bufs=3)
                            nc.vector.tensor_tensor(
                                psel[:, off:], probs[:, off:], cm_sb[:, ki, off:], ALU.mult
                            )
                            probs = psel
                        return probs, off

                    def accum_ds(probs, off, first):
                        # ds accumulates the (masked) probs at SCORE time --
                        # three tiles ahead of PV -- so the den/recip/bcast
                        # chain below overlaps the tail PV matmuls and only
                        # mult+DMA remain after the last PV lands.
                        if first:
                            nc.vector.tensor_copy(ds[:], probs[:])
                        else:
                            nc.vector.tensor_tensor(ds[:, off:], ds[:, off:],
                                                    probs[:, off:], ALU.add)

                    def emit_pv(kt2, probs, off):
                        ksb, ki = divmod(kt2, 4)
                        st = (kt2 == 0)
                        sp = (kt2 == nkt2 - 1)
                        nc.tensor.matmul(
                            ctx_ps[:, off:], v_tiles[ksb][:, ki, :], probs[:, off:],
                            start=st, stop=sp,
                        )

                    def emit_den_chain():
                        # gpsimd partition_all_reduce: den summed across
                        # partitions AND broadcast to all of them in one
                        # SBUF->SBUF op (~3.5us) -- no PE matmuls, no PSUM
                        # slots, no DRAM round trip. Then one DVE reciprocal.
                        # Emitted right after the last score so the result is
                        # ready when the last PV retires.
                        den_bc = npool.tile([128, SB], f32, tag="den_bc", name="den_bc")
                        nc.gpsimd.partition_all_reduce(den_bc[:], ds[:], 128,
                                                       bass_isa.ReduceOp.add)
                        rb_bc = npool.tile([128, SB], f32, tag="rb_bc", name="rb_bc")
                        nc.vector.reciprocal_approx_fast(rb_bc[:], den_bc[:])
                        return rb_bc

                    fifo = []
                    for k in range(min(3, nkt2)):
                        pr, off = emit_score(k)
                        accum_ds(pr, off, k == 0)
                        fifo.append((pr, off))
                    bc_s = None
                    for kt2 in range(nkt2):
                        nxt = kt2 + 3
                        if nxt < nkt2:
                            pr, off = emit_score(nxt)
                            accum_ds(pr, off, False)
                            fifo.append((pr, off))
                        if bc_s is None and nxt >= nkt2 - 1:
                            bc_s = emit_den_chain()
                        pr, off = fifo.pop(0)
                        emit_pv(kt2, pr, off)
                    ctx_sb = cpool.tile([128, SB], bft, tag="ctx_sb", name="ctx_sb")
                    nc.vector.tensor_tensor(ctx_sb[:], ctx_ps[:], bc_s[:], ALU.mult)
                    # HWDGE queues (gpsimd's SWDGE crawls at ~15-25GB/s, which
                    # delayed the AllGather start by ~40us per sb)
                    ceng = nc.sync if h % 2 == 0 else nc.scalar
                    ceng.dma_start(
                        out=cc_in[sb][h * 128:(h + 1) * 128, :], in_=ctx_sb[:]
                    )
                    # one AllGather per sb once all 4 heads' ctx is in DRAM
                    if h == QH - 1:
                        nc.gpsimd.collective_compute(
                            "AllGather",
                            ALU.bypass,
                            replica_groups=[list(range(N_CORES))],
                            ins=[cc_in[sb][:].opt()],
                            outs=[cc_out[sb][:].opt()],
                        )

            # all o_proj PE work stacked at the end: sb3's attention finishes
            # ~135us earlier, its AllGather overlaps o_proj(0..2), and the
            # scheduler fills attention-chain bubbles with o_proj matmuls
            for sb in range(NSB):
                o_proj(sb)

    nc.finalize()
    return nc


def _get_nc():
    if "nc" not in _CACHE:
        _CACHE["nc"] = _build_nc()
    return _CACHE["nc"]


def _make_in_maps(x, freqs_cos, freqs_sin, wq, bq, wk, bk, wv, bv, wo):
    x2 = np.ascontiguousarray(np.asarray(x).reshape(S, DIM))
    xT = np.ascontiguousarray(x2.T)
    # [NSB, KT//4, 128, 4, SB]: xt_p[sb, c4, p, k4, s'] = xT[128*(4c4+k4)+p, 512sb+s']
    xt_p = np.ascontiguousarray(
        xT.reshape(KT // 4, 4, 128, NSB, SB).transpose(3, 0, 2, 1, 4))
    cos = np.asarray(freqs_cos, dtype=np.float32)
    sin = np.asarray(freqs_sin, dtype=np.float32)
    def dup(t):
        return np.ascontiguousarray(np.concatenate([t, t], axis=0).astype(bf16))
    tkc_np = dup(cos.T)
    tks_np = dup(sin.T)
    jj = np.arange(SB)[None, None, :]
    pp = np.arange(128)[None, :, None]
    off = (np.arange(4) * 128)[:, None, None]
    cmask_np = np.ascontiguousarray((jj - off - pp >= 0).astype(bf16))
    wq = np.asarray(wq); wk = np.asarray(wk); wv = np.asarray(wv); wo = np.asarray(wo)
    bq = np.asarray(bq); bk = np.asarray(bk); bv = np.asarray(bv)
    in_maps = []
    for c in range(N_CORES):
        qs = slice(c * OC, (c + 1) * OC)
        ks = slice(c * HD, (c + 1) * HD)
        wqT_c = wq[qs].T.astype(bf16)   # [DIM, OC]
        wkT_c = wk[ks].T.astype(bf16)   # [DIM, HD]
        wvT_c = wv[ks].T.astype(bf16)
        woT_c = wo[qs].T.astype(bf16)

        def tile_w4(wT):
            # [DIM, O] -> [4, 128, KT//4, O]
            return np.ascontiguousarray(
                wT.reshape(4, KT // 4, 128, wT.shape[1]).transpose(0, 2, 1, 3))

        def tile_w8(wT):
            # [DIM, O] -> [8, 128, KT//8, O]
            return np.ascontiguousarray(
                wT.reshape(8, KT // 8, 128, wT.shape[1]).transpose(0, 2, 1, 3))

        def tile_wkv(wT):
            # [DIM, HD] -> [128, KT, HD]
            return np.ascontiguousarray(
                wT.reshape(KT, 128, wT.shape[1]).transpose(1, 0, 2))

        in_maps.append({
            "xt_p": xt_p,
            "wq8": tile_w8(wqT_c),
            "wk_p": tile_wkv(wkT_c),
            "wv_p": tile_wkv(wvT_c),
            "wo4": tile_w4(woT_c),
            "bqc": np.ascontiguousarray(bq[qs].astype(np.float32).reshape(QH, HD).T),
            "bkc": np.ascontiguousarray(bk[ks].astype(np.float32).reshape(1, HD).T),
            "bvc": np.ascontiguousarray(bv[ks].astype(np.float32).reshape(1, HD).T),
            "tkc": tkc_np,
            "tks": tks_np,
            "cmask": cmask_np,
            "ident": np.ascontiguousarray(np.eye(128, dtype=bf16)),
        })
    return in_maps


def _assemble(results):
    out = np.empty((S, DIM), dtype=bf16)
    for c, r in enumerate(results):
        out[:, c * OC:(c + 1) * OC] = np.asarray(r["outT"]).T
    return out.reshape(B, S, DIM)


def _mask_is_causal(mask):
    m = np.asarray(mask, dtype=np.float32)
    ii = np.arange(S, dtype=np.int64)
    expect = np.where(ii[None, :] <= ii[:, None], np.float32(0.0), np.float32(NEG))
    return m.shape == (S, S) and bool(np.array_equal(m, expect))


def _numpy_fallback(x, freqs_cos, freqs_sin, mask, wq, bq, wk, bk, wv, bv, wo):
    # exact replica of the reference in numpy (used only if mask isn't causal)
    xf = np.asarray(x).astype(np.float32).reshape(S, DIM)
    cos = np.asarray(freqs_cos, dtype=np.float32)
    sin = np.asarray(freqs_sin, dtype=np.float32)

    def tb(t):
        return np.asarray(t).astype(np.float32)

    xq = (xf @ tb(wq).T + tb(bq)).astype(bf16).astype(np.float32).reshape(S, H, HD)
    xk = (xf @ tb(wk).T + tb(bk)).astype(bf16).astype(np.float32).reshape(S, HKV, HD)
    xv = (xf @ tb(wv).T + tb(bv)).astype(bf16).astype(np.float32).reshape(S, HKV, HD)

    def rope_np(t):
        half = HD // 2
        a, b = t[..., :half], t[..., half:]
        c = cos[:, None, :]
        s = sin[:, None, :]
        return np.concatenate([a * c - b * s, a * s + b * c], axis=-1)

    xq = rope_np(xq).astype(bf16).astype(np.float32)
    xk = rope_np(xk).astype(bf16).astype(np.float32)
    key = np.repeat(xk, H // HKV, axis=1)
    val = np.repeat(xv, H // HKV, axis=1)
    scores = np.einsum("qhd,khd->hqk", xq, key) * SCALE
    scores = scores + np.asarray(mask, dtype=np.float32)[None]
    scores -= scores.max(axis=-1, keepdims=True)
    p = np.exp(scores)
    p /= p.sum(axis=-1, keepdims=True)
    ctx = np.einsum("hqk,khd->qhd", p.astype(bf16).astype(np.float32), val)
    ctx = ctx.reshape(S, H * HD).astype(bf16).astype(np.float32)
    out = (ctx @ tb(wo).T).astype(bf16)
    return out.reshape(B, S, DIM)


def kernel(x, freqs_cos, freqs_sin, mask, positions, wq, bq, wk, bk, wv, bv, wo,
           _trace=False, _tmpdir=None):
    from concourse.bass_utils import run_bass_kernel_spmd

    if not _mask_is_causal(mask):
        return _numpy_fallback(x, freqs_cos, freqs_sin, mask, wq, bq, wk, bk, wv, bv, wo)

    in_maps = _make_in_maps(x, freqs_cos, freqs_sin, wq, bq, wk, bk, wv, bv, wo)
    nc = _get_nc()
    res = run_bass_kernel_spmd(
        nc, in_maps, core_ids=list(range(N_CORES)), trace=_trace, tmpdir=_tmpdir
    )
    out = _assemble(res.results)
    if _trace:
        return out, res
    return out



# revision 47
# speedup vs baseline: 1.0684x; 1.0684x over previous
"""Trainium2 Bass kernel: GQA attention block (B=1, S=2048, DIM=4096, 32 Q / 8 KV
heads, HD=128, RoPE, causal mask, o_proj), tensor-parallel over 8 NeuronCores.

Sharding (per core c):
  - Q heads 4c..4c+3 (wq rows 512c..512c+512), KV head c (wk/wv rows 128c..).
  - x replicated; each core computes qkv projections + RoPE + causal attention
    for its heads, producing ctx^T [512 local features, 2048 seq] in bf16.
  - AllGather over the feature axis -> ctx^T full [4096, 2048], then each core
    computes o_proj for its 512 output columns (wo rows 512c..512c+512).
  - Host concatenates the per-core output column blocks.

Schedule notes (trace-driven):
  - PE is the bottleneck (~1630 512-free matmuls) and the board GPIO power
    throttle caps the sustained clock at 13/16 x 2.4GHz; idle gaps >3.4us
    additionally re-throttle to half (HAM). Goal: a gapless PE stream.
  - Phase order: [proj_k + attn_k for k=0..3] back-to-back, then ALL four
    o_proj phases at the end. Every AllGather (one 512KB op per sb, ~20-35us,
    serial on one CC stream) lands with ~100us of slack before its consumer,
    which also absorbs the 30-180us run-to-run CC stream-init stagger.
  - gpsimd carries ONLY the collective triggers (+ startup consts): anything
    else head-of-line-blocks the AG trigger behind data waits. ctx/rope-swap
    DMAs ride the two HWDGE queues; the next sb's x DMAs are pre-emitted at
    pass-B so they sit ahead of attention's DMAs in FIFO order.
  - sb0 projections run one 6-accumulator pass (q0..q3+k+v, 2 PSUM banks
    borrowed from the idle score pool) to halve startup DMA demand per cycle.
  - Softmax normalize stays on-chip: bf16 denominator matmul, DVE reciprocal,
    rank-1 PE matmul broadcasts 1/den across partitions (no DRAM round trip);
    softmax SCALE is folded into the Exp activation so q/k share rope tables.
  - Diagonal score/PV matmuls shrink their free (query) range causally.
"""

import numpy as np
import ml_dtypes

B, S, DIM = 1, 2048, 4096
H, HKV, HD = 32, 8, 128
N_CORES = 8
QH = H // N_CORES            # 4 local q heads
OC = QH * HD                 # 512 local q/out columns
SB = 512                     # seq block
NSB = S // SB                # 4
KT = DIM // 128              # 32 contraction tiles
SCALE = HD ** -0.5
NEG = -1e9

bf16 = ml_dtypes.bfloat16

_CACHE = {}


def _build_nc():
    import contextlib
    import concourse.tile as tile
    from concourse import bacc, bass_isa, library_config, mybir

    f32 = mybir.dt.float32
    bft = mybir.dt.bfloat16
    AF = mybir.ActivationFunctionType
    ALU = mybir.AluOpType

    nc = bacc.Bacc("TRN2")

    # pre-tiled on host: xt4[sb][c4] -> [128, 4, SB] contiguous; wq8[j] ->
    # [128, 4, OC] contiguous (k-tiles 4j..4j+3); wkv -> [128, KT, HD]
    xt_p = nc.declare_dram_parameter("xt_p", [NSB, KT // 4, 128, 4, SB], bft, isOutput=False)
    wq8 = nc.declare_dram_parameter("wq8", [8, 128, KT // 8, OC], bft, isOutput=False)
    wk_p = nc.declare_dram_parameter("wk_p", [128, KT, HD], bft, isOutput=False)
    wv_p = nc.declare_dram_parameter("wv_p", [128, KT, HD], bft, isOutput=False)
    wo4 = nc.declare_dram_parameter("wo4", [4, 128, KT // 4, OC], bft, isOutput=False)
    bqc = nc.declare_dram_parameter("bqc", [128, QH], mybir.dt.float32, isOutput=False)
    bkc = nc.declare_dram_parameter("bkc", [128, 1], mybir.dt.float32, isOutput=False)
    bvc = nc.declare_dram_parameter("bvc", [128, 1], mybir.dt.float32, isOutput=False)
    tkc = nc.declare_dram_parameter("tkc", [128, S], bft, isOutput=False)
    tks = nc.declare_dram_parameter("tks", [128, S], bft, isOutput=False)
    cmask = nc.declare_dram_parameter("cmask", [4, 128, SB], bft, isOutput=False)
    ident = nc.declare_dram_parameter("ident", [128, 128], bft, isOutput=False)
    outT = nc.declare_dram_parameter("outT", [OC, S], bft, isOutput=True)

    cc_warm_in = nc.dram_tensor("cc_warm_in", [1, 128], mybir.dt.float32)
    cc_warm_out = nc.dram_tensor("cc_warm_out", [N_CORES, 128], mybir.dt.float32,
                                 addr_space="Shared")
    cc_in = [nc.dram_tensor(f"cc_in{sb}", [OC, SB], bft) for sb in range(NSB)]
    # one AllGather per sb: all o_proj PE work is stacked at the END of the
    # kernel (after sb3's attention), so every gather lands with ~100us of
    # slack before its consumer -- fewer ops under the ~15-20us/op fixed cost,
    # and robust to the run-to-run CC stream-init stagger.
    cc_out = [nc.dram_tensor(f"cc_out{sb}", [N_CORES * OC, SB], bft,
                             addr_space="Shared") for sb in range(NSB)]

    with tile.TileContext(nc) as tc:
        with contextlib.ExitStack() as ctx:
            consts = ctx.enter_context(tc.tile_pool(name="consts", bufs=1))
            xpool = ctx.enter_context(tc.tile_pool(name="xpool", bufs=8))
            persist = ctx.enter_context(tc.tile_pool(name="persist", bufs=4))
            qpool = ctx.enter_context(tc.tile_pool(name="qpool", bufs=2))
            rtmp = ctx.enter_context(tc.tile_pool(name="rtmp", bufs=2))
            ppool = ctx.enter_context(tc.tile_pool(name="ppool", bufs=6))
            npool = ctx.enter_context(tc.tile_pool(name="npool", bufs=2))
            dsum = ctx.enter_context(tc.tile_pool(name="dsum", bufs=2))
            cpool = ctx.enter_context(tc.tile_pool(name="cpool", bufs=3))
            opool = ctx.enter_context(tc.tile_pool(name="opool", bufs=3))

            ps_acc = ctx.enter_context(tc.tile_pool(name="ps_acc", bufs=4, space="PSUM"))
            ps_sc = ctx.enter_context(tc.tile_pool(name="ps_sc", bufs=3, space="PSUM"))
            ps_cx = ctx.enter_context(tc.tile_pool(name="ps_cx", bufs=1, space="PSUM"))

            # tiny dummy collective, FIRST gpsimd instruction: queues behind
            # the runtime's CC init barrier so the first real AllGather starts
            # without the ~45us stream-init latency. The gathered bytes are
            # garbage and never read.
            nc.gpsimd.collective_compute(
                "AllGather", ALU.bypass,
                replica_groups=[list(range(N_CORES))],
                ins=[cc_warm_in[:].opt()],
                outs=[cc_warm_out[:].opt()],
            )
            # partition_all_reduce (softmax denominator) lives in the attn
            # gpsimd library
            nc.gpsimd.load_library(library_config.attn)

            # ---- priority DMAs: exactly what the first projection MMs need,
            # spread over FOUR trigger queues so no single queue serializes the
            # startup: sync=x stream, scalar=wq chunks, vector=wk/wv pieces
            # (+ident, then wo), gpsimd=rope tables+mask.
            KC = KT // 4
            KQ = KT // 8
            wq_ch = [None] * 8

            def load_wq_chunk(j, eng=None):
                if j < 8 and wq_ch[j] is None:
                    w = consts.tile([128, KQ, OC], bft, tag=f"wq{j}", name=f"wq{j}")
                    (eng or nc.scalar).dma_start(out=w[:], in_=wq8[j])
                    wq_ch[j] = w

            # memset for PE warmup first on the vector FIFO (no deps, fast)
            wtile = consts.tile([128, SB], bft)
            nc.vector.memset(wtile[:], 0.0)

            xt_first = xpool.tile([128, 4, SB], bft, tag="xt", name="xt")
            nc.sync.dma_start(out=xt_first[:], in_=xt_p[0, 0])
            wk_sb = consts.tile([128, KT, HD], bft)
            wv_sb = consts.tile([128, KT, HD], bft)

            def load_kv_piece(lo, hi, eng):
                eng.dma_start(out=wk_sb[:, lo:hi, :], in_=wk_p[:, lo:hi, :])
                eng.dma_start(out=wv_sb[:, lo:hi, :], in_=wv_p[:, lo:hi, :])

            # scalar = pure wq stream (4MB, chunk j lands well before its
            # k-tiles are reached); sync = x + wk/wv interleaved in
            # consumption order. ~5MB each by pass-A end.
            load_wq_chunk(0)
            load_wq_chunk(1)
            bqc_sb = consts.tile([128, QH], f32)
            nc.sync.dma_start(out=bqc_sb[:], in_=bqc[:])
            bkc_sb = consts.tile([128, 1], f32)
            nc.sync.dma_start(out=bkc_sb[:], in_=bkc[:])
            bvc_sb = consts.tile([128, 1], f32)
            nc.sync.dma_start(out=bvc_sb[:], in_=bvc[:])
            ident_sb = consts.tile([128, 128], bft)
            nc.sync.dma_start(out=ident_sb[:], in_=ident[:])
            load_kv_piece(0, 4, nc.sync)
            load_kv_piece(4, 8, nc.sync)
            # remaining wk/wv pieces interleave with sb0's x stream on sync
            # (emitted inside the c4 loop, keyed by consumption time)
            kv_pieces = {c4: [(4 * c4 + 4, 4 * c4 + 8)] for c4 in range(1, 7)}

            # PE warmup: keep the clock up while the first DMAs land. Short --
            # the first projection matmuls should be ready right after.
            warm_ps = ps_sc.tile([128, SB], f32, tag="sc_ps", name="sc_ps")
            for i in range(10):
                nc.tensor.matmul(warm_ps[:], wtile[:, 0:128], wtile[:],
                                 start=(i == 0), stop=(i == 9))

            # bulk consts on gpsimd: rope tables + mask (needed ~45us). One
            # cos/sin table pair serves q AND k -- the softmax SCALE is folded
            # into the Exp activation's scale operand instead of q's tables.
            tkc_sb = consts.tile([128, S], bft)
            nc.gpsimd.dma_start(out=tkc_sb[:], in_=tkc[:])
            tks_sb = consts.tile([128, S], bft)
            nc.gpsimd.dma_start(out=tks_sb[:], in_=tks[:])
            cm_sb = consts.tile([128, 4, SB], bft)
            nc.gpsimd.dma_start(out=cm_sb[:], in_=cmask.rearrange("j p q -> p j q"))
            # o_proj weight tiles: DMAs emitted on scalar at sb0 pass B (behind
            # the projection weights, ahead of sb1's x stream; needed ~150us)
            wo_ch = [consts.tile([128, KC, OC], bft, tag=f"wo{j}", name=f"wo{j}")
                     for j in range(4)]


            def wq_at(kt, lo, hi):
                return wq_ch[kt // KQ][:, kt % KQ, lo:hi]

            def wo_at(kt, lo, hi):
                return wo_ch[kt // KC][:, kt % KC, lo:hi]

            # persistent per-sb K^T and V tiles
            k_tiles = [None] * NSB   # [128 d, SB s] bf16
            v_tiles = [None] * NSB   # [128 s, 4, 128 d] bf16

            rope_ctr = [0]

            def rope(dst, src, tcos, tsin, s0):
                # dst/src: [128, SB] bf16. tcos/tsin have cos/sin duplicated in
                # both partition halves. swp = src with halves swapped (DMA
                # partition move), so every DVE op is partition-aligned.
                # Swap DMAs ride the HWDGE queues (alternating): gpsimd must
                # stay empty so AllGather triggers fire the moment ctx lands
                # (the tile scheduler queues swp DMAs ahead of AG triggers,
                # and a swp head-of-line-blocks until its qraw exists).
                cL = tcos[0:64, s0:s0 + SB]
                cH = tcos[64:128, s0:s0 + SB]
                sL = tsin[0:64, s0:s0 + SB]
                sH = tsin[64:128, s0:s0 + SB]
                swp = rtmp.tile([128, SB], bft, tag="ropeswp", name="ropeswp")
                seng = nc.sync if rope_ctr[0] % 2 == 0 else nc.scalar
                rope_ctr[0] += 1
                seng.dma_start(out=swp[0:64, :], in_=src[64:128, :])
                seng.dma_start(out=swp[64:128, :], in_=src[0:64, :])
                tA = rtmp.tile([128, SB], bft, tag="ropetA", name="ropetA")
                tB = rtmp.tile([128, SB], bft, tag="ropetB", name="ropetB")
                nc.vector.tensor_tensor(tA[0:64, :], src[0:64, :], cL, ALU.mult)
                nc.vector.tensor_tensor(tA[64:128, :], swp[64:128, :], sH, ALU.mult)
                nc.vector.tensor_tensor(tB[0:64, :], swp[0:64, :], sL, ALU.mult)
                nc.vector.tensor_tensor(tB[64:128, :], src[64:128, :], cH, ALU.mult)
                nc.vector.tensor_tensor(dst[0:64, :], tA[0:64, :], tB[0:64, :], ALU.subtract)
                nc.vector.tensor_tensor(dst[64:128, :], tA[64:128, :], tB[64:128, :], ALU.add)

            def o_proj(sb):
                s0 = sb * SB
                o_ps = [ps_acc.tile([128, SB], f32, tag="acc", name="acc") for _ in range(QH)]
                # cc_out rows: core*512 + head*128 + p, so chunk feature-tile
                # b = c4*4+k4 maps 1:1 to the wo contraction tile index
                co_r = cc_out[sb].rearrange("(c b p) s -> c p b s", p=128, b=4)
                for c4 in range(8):
                    rt4 = opool.tile([128, 4, SB], bft, tag="rt", name="rt", bufs=4)
                    eng = nc.sync if c4 % 2 == 0 else nc.scalar
                    eng.dma_start(out=rt4[:], in_=co_r[c4])
                    for k4 in range(4):
                        ft = c4 * 4 + k4
                        for ct in range(QH):
                            nc.tensor.matmul(
                                o_ps[ct][:], wo_at(ft, ct * 128, (ct + 1) * 128),
                                rt4[:, k4, :],
                                start=(ft == 0), stop=(ft == KT - 1),
                            )
                for ct in range(QH):
                    ot = opool.tile([128, SB], bft, tag="ot", name="ot")
                    nc.vector.tensor_copy(ot[:], o_ps[ct][:])
                    nc.scalar.dma_start(
                        out=outT[ct * 128:(ct + 1) * 128, s0:s0 + SB], in_=ot[:]
                    )

            xt_next = [None] * (KT // 4)
            for sb in range(NSB):
                s0 = sb * SB
                q_sb = [None] * QH

                # ---- projection pass A ----
                # sb0 runs a SINGLE 6-accumulator pass (q0..q3 + k + v): two
                # extra accumulators borrowed from the idle score pool (no
                # attention overlaps sb0's projections). 6 MMs per k-tile
                # halves the startup DMA demand per PE-cycle, which is what
                # the two HWDGE queues can actually sustain while weights and
                # x stream in cold.
                na = QH if sb == 0 else 2
                qa_ps = [ps_acc.tile([128, SB], f32, tag="acc", name="acc") for _ in range(2)]
                if sb == 0:
                    qa_ps += [ps_sc.tile([128, SB], f32, tag="sc_ps", name="sc_ps")
                              for _ in range(2)]
                k_ps = ps_acc.tile([128, SB], f32, tag="acc", name="acc")
                v_ps = ps_acc.tile([128, SB], f32, tag="acc", name="acc")
                xt_chunks = [None] * (KT // 4)
                for c4 in range(KT // 4):
                    if sb == 0:
                        if c4 == 0:
                            xt4 = xt_first
                        else:
                            # sb0's x all on sync: scalar stays a pure wq
                            # stream so neither starves the cold ramp
                            xt4 = xpool.tile([128, 4, SB], bft, tag="xt", name="xt")
                            nc.sync.dma_start(out=xt4[:], in_=xt_p[sb, c4])
                        load_wq_chunk(c4 + 2)
                        for lo, hi in kv_pieces.get(c4, []):
                            load_kv_piece(lo, hi, nc.sync)
                    else:
                        # pre-emitted at the previous sb's pass-B start
                        xt4 = xt_next[c4]
                    xt_chunks[c4] = xt4
                    for k4 in range(4):
                        kt = c4 * 4 + k4
                        xt = xt4[:, k4, :]
                        st = (kt == 0)
                        sp = (kt == KT - 1)
                        for h in range(na):
                            nc.tensor.matmul(
                                qa_ps[h][:], wq_at(kt, h * 128, (h + 1) * 128), xt,
                                start=st, stop=sp,
                            )
                        nc.tensor.matmul(k_ps[:], wk_sb[:, kt, :], xt, start=st, stop=sp)
                        nc.tensor.matmul(v_ps[:], wv_sb[:, kt, :], xt, start=st, stop=sp)

                if sb == 0:
                    # sb0's chunks are fully read now: pre-emit sb1's x DMAs
                    # (slots recycle progressively), then wo on scalar (which
                    # is idle after the wq stream; needed only at ~300us)
                    for c4 in range(KT // 4):
                        t = xpool.tile([128, 4, SB], bft, tag="xt", name="xt")
                        eng = nc.sync if c4 % 2 == 0 else nc.scalar
                        eng.dma_start(out=t[:], in_=xt_p[1, c4])
                        xt_next[c4] = t
                    for j in range(4):
                        nc.scalar.dma_start(out=wo_ch[j][:], in_=wo4[j])

                for h in range(na):
                    qraw = qpool.tile([128, SB], bft, tag="qraw", name="qraw")
                    nc.scalar.activation(qraw[:], qa_ps[h][:], AF.Identity,
                                         bias=bqc_sb[:, h:h + 1])
                    qr = qpool.tile([128, SB], bft, tag="qrope", name="qrope", bufs=8)
                    rope(qr, qraw, tkc_sb, tks_sb, s0)
                    q_sb[h] = qr
                kraw = qpool.tile([128, SB], bft, tag="kraw", name="kraw")
                nc.scalar.activation(kraw[:], k_ps[:], AF.Identity,
                                     bias=bkc_sb[:, 0:1])
                k_t = persist.tile([128, SB], bft, tag="k_t", name="k_t")
                rope(k_t, kraw, tkc_sb, tks_sb, s0)
                k_tiles[sb] = k_t
                vraw = qpool.tile([128, SB], bft, tag="vraw", name="vraw")
                nc.scalar.activation(vraw[:], v_ps[:], AF.Identity,
                                     bias=bvc_sb[:, 0:1])
                # transpose V on the PE (vraw_tile.T @ I) -- DMA_TRANSPOSE
                # serializes behind in-flight collectives, this doesn't
                vt_ps = ps_sc.tile([128, QH, 128], f32, tag="sc_ps", name="sc_ps")
                for i in range(QH):
                    nc.tensor.matmul(vt_ps[:, i, :], vraw[:, i * 128:(i + 1) * 128],
                                     ident_sb[:], start=True, stop=True)
                v_t = persist.tile([128, QH, 128], bft, tag="v_t", name="v_t")
                nc.vector.tensor_copy(v_t[:], vt_ps[:])
                v_tiles[sb] = v_t

                # ---- projection pass B (sb>0): q2, q3, x chunks resident.
                # Forward c4 order so chunk slots free progressively.
                if sb > 0:
                    qb_ps = [ps_acc.tile([128, SB], f32, tag="acc", name="acc") for _ in range(2)]
                    first_b = True
                    for c4 in range(KT // 4):
                        xt4 = xt_chunks[c4]
                        for k4 in range(4):
                            kt = c4 * 4 + k4
                            xt = xt4[:, k4, :]
                            last_b = (c4 == KT // 4 - 1 and k4 == 3)
                            for h in range(2):
                                nc.tensor.matmul(
                                    qb_ps[h][:], wq_at(kt, (2 + h) * 128, (3 + h) * 128), xt,
                                    start=first_b, stop=last_b,
                                )
                            first_b = False
                    # pre-emit the NEXT sb's x DMAs (after pass B's reads so
                    # the WAR slot reuse is tracked; the DMAs still fire
                    # during pass B execution, ahead of this sb's ctx DMAs)
                    if sb + 1 < NSB:
                        for c4 in range(KT // 4):
                            t = xpool.tile([128, 4, SB], bft, tag="xt", name="xt")
                            eng = nc.sync if c4 % 2 == 0 else nc.scalar
                            eng.dma_start(out=t[:], in_=xt_p[sb + 1, c4])
                            xt_next[c4] = t
                    for h in range(2):
                        qraw = qpool.tile([128, SB], bft, tag="qraw", name="qraw")
                        nc.scalar.activation(qraw[:], qb_ps[h][:], AF.Identity,
                                             bias=bqc_sb[:, 2 + h:3 + h])
                        qr = qpool.tile([128, SB], bft, tag="qrope", name="qrope", bufs=8)
                        rope(qr, qraw, tkc_sb, tks_sb, s0)
                        q_sb[2 + h] = qr

                # ---- causal attention for q-block sb, 4 heads ----
                nkt2 = 4 * (sb + 1)
                for h in range(QH):
                    # sb3 (the longest attention block, no projection filler
                    # after it): every head gets its OWN ctx bank from ps_acc
                    # (free: no proj interleaves behind attn3), removing the
                    # per-head serialization through the single ps_cx bank.
                    if sb == 3:
                        ctx_ps = ps_acc.tile([128, SB], f32, tag="acc", name="acc")
                    else:
                        ctx_ps = ps_cx.tile([128, SB], f32, tag="ctx_ps", name="ctx_ps")

                    ds = dsum.tile([128, SB], f32, tag="ds", name="ds")

                    def emit_score(kt2):
                        ksb, ki = divmod(kt2, 4)
                        off = 128 * ki if ksb == sb else 0
                        sc_ps = ps_sc.tile([128, SB], f32, tag="sc_ps", name="sc_ps")
                        nc.tensor.matmul(
                            sc_ps[:, off:],
                            k_tiles[ksb][:, ki * 128:(ki + 1) * 128],
                            q_sb[h][:, off:],
                            start=True, stop=True,
                        )
                        probs = ppool.tile([128, SB], bft, tag="probs", name="probs", bufs=7)
                        nc.scalar.activation(probs[:, off:], sc_ps[:, off:], AF.Exp,
                                             scale=float(SCALE))
                        if ksb == sb:
                            psel = ppool.tile([128, SB], bft, tag="psel", name="psel", bufs=3)
                            nc.vector.tensor_tensor(
                                psel[:, off:], probs[:, off:], cm_sb[:, ki, off:], ALU.mult
                            )
                            probs = psel
                        return probs, off

                    def accum_ds(probs, off, first):
                        # ds accumulates the (masked) probs at SCORE time --
                        # three tiles ahead of PV -- so the den/recip/bcast
                        # chain below overlaps the tail PV matmuls and only
                        # mult+DMA remain after the last PV lands.
                        if first:
                            nc.vector.tensor_copy(ds[:], probs[:])
                        else:
                            nc.vector.tensor_tensor(ds[:, off:], ds[:, off:],
                                                    probs[:, off:], ALU.add)

                    def emit_pv(kt2, probs, off):
                        ksb, ki = divmod(kt2, 4)
                        st = (kt2 == 0)
                        sp = (kt2 == nkt2 - 1)
                        nc.tensor.matmul(
                            ctx_ps[:, off:], v_tiles[ksb][:, ki, :], probs[:, off:],
                            start=st, stop=sp,
                        )

                    def emit_den_chain():
                        # gpsimd partition_all_reduce: den summed across
                        # partitions AND broadcast to all of them in one
                        # SBUF->SBUF op (~3.5us) -- no PE matmuls, no PSUM
                        # slots, no DRAM round trip. Then one DVE reciprocal.
                        # Emitted right after the last score so the result is
                        # ready when the last PV retires.
                        den_bc = npool.tile([128, SB], f32, tag="den_bc", name="den_bc")
                        nc.gpsimd.partition_all_reduce(den_bc[:], ds[:], 128,
                                                       bass_isa.ReduceOp.add)
                        rb_bc = npool.tile([128, SB], f32, tag="rb_bc", name="rb_bc")
                        nc.vector.reciprocal_approx_fast(rb_bc[:], den_bc[:])
                        return rb_bc

                    fifo = []
                    for k in range(min(3, nkt2)):
                        pr, off = emit_score(k)
                        accum_ds(pr, off, k == 0)
                        fifo.append((pr, off))
                    bc_s = None
                    for kt2 in range(nkt2):
                        nxt = kt2 + 3
                        if nxt < nkt2:
                            pr, off = emit_score(nxt)
                            accum_ds(pr, off, False)
                            fifo.append((pr, off))
                        if bc_s is None and nxt >= nkt2 - 1:
                            bc_s = emit_den_chain()
                        pr, off = fifo.pop(0)
                        emit_pv(kt2, pr, off)
                    ctx_sb = cpool.tile([128, SB], bft, tag="ctx_sb", name="ctx_sb")
                    nc.vector.tensor_tensor(ctx_sb[:], ctx_ps[:], bc_s[:], ALU.mult)
                    # HWDGE queues (gpsimd's SWDGE crawls at ~15-25GB/s, which
                    # delayed the AllGather start by ~40us per sb)
                    ceng = nc.sync if h % 2 == 0 else nc.scalar
                    ceng.dma_start(
                        out=cc_in[sb][h * 128:(h + 1) * 128, :], in_=ctx_sb[:]
                    )
                    # one AllGather per sb once all 4 heads' ctx is in DRAM
                    if h == QH - 1:
                        nc.gpsimd.collective_compute(
                            "AllGather",
                            ALU.bypass,
                            replica_groups=[list(range(N_CORES))],
                            ins=[cc_in[sb][:].opt()],
                            outs=[cc_out[sb][:].opt()],
                        )

            # all o_proj PE work stacked at the end: sb3's attention finishes
            # ~135us earlier, its AllGather overlaps o_proj(0..2), and the
            # scheduler fills attention-chain bubbles with o_proj matmuls
            for sb in range(NSB):
                o_proj(sb)

    nc.finalize()
    return nc


def _get_nc():
    if "nc" not in _CACHE:
        _CACHE["nc"] = _build_nc()
    return _CACHE["nc"]


def _make_in_maps(x, freqs_cos, freqs_sin, wq, bq, wk, bk, wv, bv, wo):
    x2 = np.ascontiguousarray(np.asarray(x).reshape(S, DIM))
    xT = np.ascontiguousarray(x2.T)
    # [NSB, KT//4, 128, 4, SB]: xt_p[sb, c4, p, k4, s'] = xT[128*(4c4+k4)+p, 512sb+s']
    xt_p = np.ascontiguousarray(
        xT.reshape(KT // 4, 4, 128, NSB, SB).transpose(3, 0, 2, 1, 4))
    cos = np.asarray(freqs_cos, dtype=np.float32)
    sin = np.asarray(freqs_sin, dtype=np.float32)
    def dup(t):
        return np.ascontiguousarray(np.concatenate([t, t], axis=0).astype(bf16))
    tkc_np = dup(cos.T)
    tks_np = dup(sin.T)
    jj = np.arange(SB)[None, None, :]
    pp = np.arange(128)[None, :, None]
    off = (np.arange(4) * 128)[:, None, None]
    cmask_np = np.ascontiguousarray((jj - off - pp >= 0).astype(bf16))
    wq = np.asarray(wq); wk = np.asarray(wk); wv = np.asarray(wv); wo = np.asarray(wo)
    bq = np.asarray(bq); bk = np.asarray(bk); bv = np.asarray(bv)
    in_maps = []
    for c in range(N_CORES):
        qs = slice(c * OC, (c + 1) * OC)
        ks = slice(c * HD, (c + 1) * HD)
        wqT_c = wq[qs].T.astype(bf16)   # [DIM, OC]
        wkT_c = wk[ks].T.astype(bf16)   # [DIM, HD]
        wvT_c = wv[ks].T.astype(bf16)
        woT_c = wo[qs].T.astype(bf16)

        def tile_w4(wT):
            # [DIM, O] -> [4, 128, KT//4, O]
            return np.ascontiguousarray(
                wT.reshape(4, KT // 4, 128, wT.shape[1]).transpose(0, 2, 1, 3))

        def tile_w8(wT):
            # [DIM, O] -> [8, 128, KT//8, O]
            return np.ascontiguousarray(
                wT.reshape(8, KT // 8, 128, wT.shape[1]).transpose(0, 2, 1, 3))

        def tile_wkv(wT):
            # [DIM, HD] -> [128, KT, HD]
            return np.ascontiguousarray(
                wT.reshape(KT, 128, wT.shape[1]).transpose(1, 0, 2))

        in_maps.append({
            "xt_p": xt_p,
            "wq8": tile_w8(wqT_c),
            "wk_p": tile_wkv(wkT_c),
            "wv_p": tile_wkv(wvT_c),
            "wo4": tile_w4(woT_c),
            "bqc": np.ascontiguousarray(bq[qs].astype(np.float32).reshape(QH, HD).T),
            "bkc": np.ascontiguousarray(bk[ks].astype(np.float32).reshape(1, HD).T),
            "bvc": np.ascontiguousarray(bv[ks].astype(np.float32).reshape(1, HD).T),
            "tkc": tkc_np,
            "tks": tks_np,
            "cmask": cmask_np,
            "ident": np.ascontiguousarray(np.eye(128, dtype=bf16)),
        })
    return in_maps


def _assemble(results):
    out = np.empty((S, DIM), dtype=bf16)
    for c, r in enumerate(results):
        out[:, c * OC:(c + 1) * OC] = np.asarray(r["outT"]).T
    return out.reshape(B, S, DIM)


def _mask_is_causal(mask):
    m = np.asarray(mask, dtype=np.float32)
    ii = np.arange(S, dtype=np.int64)
    expect = np.where(ii[None, :] <= ii[:, None], np.float32(0.0), np.float32(NEG))
    return m.shape == (S, S) and bool(np.array_equal(m, expect))


def _numpy_fallback(x, freqs_cos, freqs_sin, mask, wq, bq, wk, bk, wv, bv, wo):
    # exact replica of the reference in numpy (used only if mask isn't causal)
    xf = np.asarray(x).astype(np.float32).reshape(S, DIM)
    cos = np.asarray(freqs_cos, dtype=np.float32)
    sin = np.asarray(freqs_sin, dtype=np.float32)

    def tb(t):
        return np.asarray(t).astype(np.float32)

    xq = (xf @ tb(wq).T + tb(bq)).astype(bf16).astype(np.float32).reshape(S, H, HD)
    xk = (xf @ tb(wk).T + tb(bk)).astype(bf16).astype(np.float32).reshape(S, HKV, HD)
    xv = (xf @ tb(wv).T + tb(bv)).astype(bf16).astype(np.float32).reshape(S, HKV, HD)

    def rope_np(t):
        half = HD // 2
        a, b = t[..., :half], t[..., half:]
        c = cos[:, None, :]
        s = sin[:, None, :]
        return np.concatenate([a * c - b * s, a * s + b * c], axis=-1)

    xq = rope_np(xq).astype(bf16).astype(np.float32)
    xk = rope_np(xk).astype(bf16).astype(np.float32)
    key = np.repeat(xk, H // HKV, axis=1)
    val = np.repeat(xv, H // HKV, axis=1)
    scores = np.einsum("qhd,khd->hqk", xq, key) * SCALE
    scores = scores + np.asarray(mask, dtype=np.float32)[None]
    scores -= scores.max(axis=-1, keepdims=True)
    p = np.exp(scores)
    p /= p.sum(axis=-1, keepdims=True)
    ctx = np.einsum("hqk,khd->qhd", p.astype(bf16).astype(np.float32), val)
    ctx = ctx.reshape(S, H * HD).astype(bf16).astype(np.float32)
    out = (ctx @ tb(wo).T).astype(bf16)
    return out.reshape(B, S, DIM)


def kernel(x, freqs_cos, freqs_sin, mask, positions, wq, bq, wk, bk, wv, bv, wo,
           _trace=False, _tmpdir=None):
    from concourse.bass_utils import run_bass_kernel_spmd

    if not _mask_is_causal(mask):
        return _numpy_fallback(x, freqs_cos, freqs_sin, mask, wq, bq, wk, bk, wv, bv, wo)

    in_maps = _make_in_maps(x, freqs_cos, freqs_sin, wq, bq, wk, bk, wv, bv, wo)
    nc = _get_nc()
    res = run_bass_kernel_spmd(
        nc, in_maps, core_ids=list(range(N_CORES)), trace=_trace, tmpdir=_tmpdir
    )
    out = _assemble(res.results)
    if _trace:
        return out, res
    return out



# revision 48
# speedup vs baseline: 1.0791x; 1.0100x over previous
"""Trainium2 Bass kernel: GQA attention block (B=1, S=2048, DIM=4096, 32 Q / 8 KV
heads, HD=128, RoPE, causal mask, o_proj), tensor-parallel over 8 NeuronCores.

Sharding (per core c):
  - Q heads 4c..4c+3 (wq rows 512c..512c+512), KV head c (wk/wv rows 128c..).
  - x replicated; each core computes qkv projections + RoPE + causal attention
    for its heads, producing ctx^T [512 local features, 2048 seq] in bf16.
  - AllGather over the feature axis -> ctx^T full [4096, 2048], then each core
    computes o_proj for its 512 output columns (wo rows 512c..512c+512).
  - Host concatenates the per-core output column blocks.

Schedule notes (trace-driven):
  - PE is the bottleneck (~1630 512-free matmuls) and the board GPIO power
    throttle caps the sustained clock at 13/16 x 2.4GHz; idle gaps >3.4us
    additionally re-throttle to half (HAM). Goal: a gapless PE stream.
  - Phase order: [proj_k + attn_k for k=0..3] back-to-back, then ALL four
    o_proj phases at the end. Every AllGather (one 512KB op per sb, ~20-35us,
    serial on one CC stream) lands with ~100us of slack before its consumer,
    which also absorbs the 30-180us run-to-run CC stream-init stagger.
  - gpsimd carries ONLY the collective triggers (+ startup consts): anything
    else head-of-line-blocks the AG trigger behind data waits. ctx/rope-swap
    DMAs ride the two HWDGE queues; the next sb's x DMAs are pre-emitted at
    pass-B so they sit ahead of attention's DMAs in FIFO order.
  - sb0 projections run one 6-accumulator pass (q0..q3+k+v, 2 PSUM banks
    borrowed from the idle score pool) to halve startup DMA demand per cycle.
  - Softmax normalize stays on-chip: bf16 denominator matmul, DVE reciprocal,
    rank-1 PE matmul broadcasts 1/den across partitions (no DRAM round trip);
    softmax SCALE is folded into the Exp activation so q/k share rope tables.
  - Diagonal score/PV matmuls shrink their free (query) range causally.
"""

import numpy as np
import ml_dtypes

B, S, DIM = 1, 2048, 4096
H, HKV, HD = 32, 8, 128
N_CORES = 8
QH = H // N_CORES            # 4 local q heads
OC = QH * HD                 # 512 local q/out columns
SB = 512                     # seq block
NSB = S // SB                # 4
KT = DIM // 128              # 32 contraction tiles
SCALE = HD ** -0.5
NEG = -1e9

bf16 = ml_dtypes.bfloat16

_CACHE = {}


def _build_nc():
    import contextlib
    import concourse.tile as tile
    from concourse import bacc, bass_isa, library_config, mybir

    f32 = mybir.dt.float32
    bft = mybir.dt.bfloat16
    AF = mybir.ActivationFunctionType
    ALU = mybir.AluOpType

    nc = bacc.Bacc("TRN2")

    # pre-tiled on host: xt4[sb][c4] -> [128, 4, SB] contiguous; wq8[j] ->
    # [128, 4, OC] contiguous (k-tiles 4j..4j+3); wkv -> [128, KT, HD]
    xt_p = nc.declare_dram_parameter("xt_p", [NSB, KT // 4, 128, 4, SB], bft, isOutput=False)
    wq8 = nc.declare_dram_parameter("wq8", [8, 128, KT // 8, OC], bft, isOutput=False)
    wk_p = nc.declare_dram_parameter("wk_p", [128, KT, HD], bft, isOutput=False)
    wv_p = nc.declare_dram_parameter("wv_p", [128, KT, HD], bft, isOutput=False)
    wo4 = nc.declare_dram_parameter("wo4", [4, 128, KT // 4, OC], bft, isOutput=False)
    bqc = nc.declare_dram_parameter("bqc", [128, QH], mybir.dt.float32, isOutput=False)
    bkc = nc.declare_dram_parameter("bkc", [128, 1], mybir.dt.float32, isOutput=False)
    bvc = nc.declare_dram_parameter("bvc", [128, 1], mybir.dt.float32, isOutput=False)
    tkc = nc.declare_dram_parameter("tkc", [128, S], bft, isOutput=False)
    tks = nc.declare_dram_parameter("tks", [128, S], bft, isOutput=False)
    cmask = nc.declare_dram_parameter("cmask", [4, 128, SB], bft, isOutput=False)
    ident = nc.declare_dram_parameter("ident", [128, 128], bft, isOutput=False)
    outT = nc.declare_dram_parameter("outT", [OC, S], bft, isOutput=True)

    cc_warm_in = nc.dram_tensor("cc_warm_in", [1, 128], mybir.dt.float32)
    cc_warm_out = nc.dram_tensor("cc_warm_out", [N_CORES, 128], mybir.dt.float32,
                                 addr_space="Shared")
    cc_in = [nc.dram_tensor(f"cc_in{sb}", [OC, SB], bft) for sb in range(NSB)]
    # one AllGather per sb: all o_proj PE work is stacked at the END of the
    # kernel (after sb3's attention), so every gather lands with ~100us of
    # slack before its consumer -- fewer ops under the ~15-20us/op fixed cost,
    # and robust to the run-to-run CC stream-init stagger.
    cc_out = [nc.dram_tensor(f"cc_out{sb}", [N_CORES * OC, SB], bft,
                             addr_space="Shared") for sb in range(NSB)]

    with tile.TileContext(nc) as tc:
        with contextlib.ExitStack() as ctx:
            consts = ctx.enter_context(tc.tile_pool(name="consts", bufs=1))
            xpool = ctx.enter_context(tc.tile_pool(name="xpool", bufs=8))
            persist = ctx.enter_context(tc.tile_pool(name="persist", bufs=4))
            qpool = ctx.enter_context(tc.tile_pool(name="qpool", bufs=2))
            rtmp = ctx.enter_context(tc.tile_pool(name="rtmp", bufs=2))
            ppool = ctx.enter_context(tc.tile_pool(name="ppool", bufs=6))
            npool = ctx.enter_context(tc.tile_pool(name="npool", bufs=2))
            dsum = ctx.enter_context(tc.tile_pool(name="dsum", bufs=2))
            cpool = ctx.enter_context(tc.tile_pool(name="cpool", bufs=3))
            opool = ctx.enter_context(tc.tile_pool(name="opool", bufs=3))

            ps_acc = ctx.enter_context(tc.tile_pool(name="ps_acc", bufs=4, space="PSUM"))
            ps_sc = ctx.enter_context(tc.tile_pool(name="ps_sc", bufs=3, space="PSUM"))
            ps_cx = ctx.enter_context(tc.tile_pool(name="ps_cx", bufs=1, space="PSUM"))

            # tiny dummy collective, FIRST gpsimd instruction: queues behind
            # the runtime's CC init barrier so the first real AllGather starts
            # without the ~45us stream-init latency. The gathered bytes are
            # garbage and never read.
            nc.gpsimd.collective_compute(
                "AllGather", ALU.bypass,
                replica_groups=[list(range(N_CORES))],
                ins=[cc_warm_in[:].opt()],
                outs=[cc_warm_out[:].opt()],
            )
            # partition_all_reduce (softmax denominator) lives in the attn
            # gpsimd library
            nc.gpsimd.load_library(library_config.attn)

            # ---- priority DMAs: exactly what the first projection MMs need,
            # spread over FOUR trigger queues so no single queue serializes the
            # startup: sync=x stream, scalar=wq chunks, vector=wk/wv pieces
            # (+ident, then wo), gpsimd=rope tables+mask.
            KC = KT // 4
            KQ = KT // 8
            wq_ch = [None] * 8

            def load_wq_chunk(j, eng=None):
                if j < 8 and wq_ch[j] is None:
                    w = consts.tile([128, KQ, OC], bft, tag=f"wq{j}", name=f"wq{j}")
                    (eng or nc.scalar).dma_start(out=w[:], in_=wq8[j])
                    wq_ch[j] = w

            # memset for PE warmup first on the vector FIFO (no deps, fast)
            wtile = consts.tile([128, SB], bft)
            nc.vector.memset(wtile[:], 0.0)

            xt_first = xpool.tile([128, 4, SB], bft, tag="xt", name="xt")
            nc.sync.dma_start(out=xt_first[:], in_=xt_p[0, 0])
            wk_sb = consts.tile([128, KT, HD], bft)
            wv_sb = consts.tile([128, KT, HD], bft)

            def load_kv_piece(lo, hi, eng):
                eng.dma_start(out=wk_sb[:, lo:hi, :], in_=wk_p[:, lo:hi, :])
                eng.dma_start(out=wv_sb[:, lo:hi, :], in_=wv_p[:, lo:hi, :])

            # scalar = pure wq stream (4MB, chunk j lands well before its
            # k-tiles are reached); sync = x + wk/wv interleaved in
            # consumption order. ~5MB each by pass-A end.
            load_wq_chunk(0)
            load_wq_chunk(1)
            bqc_sb = consts.tile([128, QH], f32)
            nc.sync.dma_start(out=bqc_sb[:], in_=bqc[:])
            bkc_sb = consts.tile([128, 1], f32)
            nc.sync.dma_start(out=bkc_sb[:], in_=bkc[:])
            bvc_sb = consts.tile([128, 1], f32)
            nc.sync.dma_start(out=bvc_sb[:], in_=bvc[:])
            ident_sb = consts.tile([128, 128], bft)
            nc.sync.dma_start(out=ident_sb[:], in_=ident[:])
            load_kv_piece(0, 4, nc.sync)
            load_kv_piece(4, 8, nc.sync)
            # remaining wk/wv pieces interleave with sb0's x stream on sync
            # (emitted inside the c4 loop, keyed by consumption time)
            kv_pieces = {c4: [(4 * c4 + 4, 4 * c4 + 8)] for c4 in range(1, 7)}

            # PE warmup: keep the clock up while the first DMAs land. Short --
            # the first projection matmuls should be ready right after.
            warm_ps = ps_sc.tile([128, SB], f32, tag="sc_ps", name="sc_ps")
            for i in range(10):
                nc.tensor.matmul(warm_ps[:], wtile[:, 0:128], wtile[:],
                                 start=(i == 0), stop=(i == 9))

            # bulk consts on gpsimd: rope tables + mask (needed ~45us). One
            # cos/sin table pair serves q AND k -- the softmax SCALE is folded
            # into the Exp activation's scale operand instead of q's tables.
            tkc_sb = consts.tile([128, S], bft)
            nc.gpsimd.dma_start(out=tkc_sb[:], in_=tkc[:])
            tks_sb = consts.tile([128, S], bft)
            nc.gpsimd.dma_start(out=tks_sb[:], in_=tks[:])
            cm_sb = consts.tile([128, 4, SB], bft)
            nc.gpsimd.dma_start(out=cm_sb[:], in_=cmask.rearrange("j p q -> p j q"))
            # o_proj weight tiles: DMAs emitted on scalar at sb0 pass B (behind
            # the projection weights, ahead of sb1's x stream; needed ~150us)
            wo_ch = [consts.tile([128, KC, OC], bft, tag=f"wo{j}", name=f"wo{j}")
                     for j in range(4)]


            def wq_at(kt, lo, hi):
                return wq_ch[kt // KQ][:, kt % KQ, lo:hi]

            def wo_at(kt, lo, hi):
                return wo_ch[kt // KC][:, kt % KC, lo:hi]

            # persistent per-sb K^T and V tiles
            k_tiles = [None] * NSB   # [128 d, SB s] bf16
            v_tiles = [None] * NSB   # [128 s, 4, 128 d] bf16

            rope_ctr = [0]

            def rope(dst, src, tcos, tsin, s0):
                # dst/src: [128, SB] bf16. tcos/tsin have cos/sin duplicated in
                # both partition halves. swp = src with halves swapped (DMA
                # partition move), so every DVE op is partition-aligned.
                # Swap DMAs ride the HWDGE queues (alternating): gpsimd must
                # stay empty so AllGather triggers fire the moment ctx lands
                # (the tile scheduler queues swp DMAs ahead of AG triggers,
                # and a swp head-of-line-blocks until its qraw exists).
                cL = tcos[0:64, s0:s0 + SB]
                cH = tcos[64:128, s0:s0 + SB]
                sL = tsin[0:64, s0:s0 + SB]
                sH = tsin[64:128, s0:s0 + SB]
                swp = rtmp.tile([128, SB], bft, tag="ropeswp", name="ropeswp")
                seng = nc.sync if rope_ctr[0] % 2 == 0 else nc.scalar
                rope_ctr[0] += 1
                seng.dma_start(out=swp[0:64, :], in_=src[64:128, :])
                seng.dma_start(out=swp[64:128, :], in_=src[0:64, :])
                tA = rtmp.tile([128, SB], bft, tag="ropetA", name="ropetA")
                tB = rtmp.tile([128, SB], bft, tag="ropetB", name="ropetB")
                nc.vector.tensor_tensor(tA[0:64, :], src[0:64, :], cL, ALU.mult)
                nc.vector.tensor_tensor(tA[64:128, :], swp[64:128, :], sH, ALU.mult)
                nc.vector.tensor_tensor(tB[0:64, :], swp[0:64, :], sL, ALU.mult)
                nc.vector.tensor_tensor(tB[64:128, :], src[64:128, :], cH, ALU.mult)
                nc.vector.tensor_tensor(dst[0:64, :], tA[0:64, :], tB[0:64, :], ALU.subtract)
                nc.vector.tensor_tensor(dst[64:128, :], tA[64:128, :], tB[64:128, :], ALU.add)

            def o_proj(sb):
                s0 = sb * SB
                o_ps = [ps_acc.tile([128, SB], f32, tag="acc", name="acc") for _ in range(QH)]
                # cc_out rows: core*512 + head*128 + p, so chunk feature-tile
                # b = c4*4+k4 maps 1:1 to the wo contraction tile index
                co_r = cc_out[sb].rearrange("(c b p) s -> c p b s", p=128, b=4)
                for c4 in range(8):
                    rt4 = opool.tile([128, 4, SB], bft, tag="rt", name="rt", bufs=4)
                    eng = nc.sync if c4 % 2 == 0 else nc.scalar
                    eng.dma_start(out=rt4[:], in_=co_r[c4])
                    for k4 in range(4):
                        ft = c4 * 4 + k4
                        for ct in range(QH):
                            nc.tensor.matmul(
                                o_ps[ct][:], wo_at(ft, ct * 128, (ct + 1) * 128),
                                rt4[:, k4, :],
                                start=(ft == 0), stop=(ft == KT - 1),
                            )
                for ct in range(QH):
                    ot = opool.tile([128, SB], bft, tag="ot", name="ot")
                    nc.vector.tensor_copy(ot[:], o_ps[ct][:])
                    nc.scalar.dma_start(
                        out=outT[ct * 128:(ct + 1) * 128, s0:s0 + SB], in_=ot[:]
                    )

            xt_next = [None] * (KT // 4)
            for sb in range(NSB):
                s0 = sb * SB
                q_sb = [None] * QH

                # ---- projection pass A ----
                # sb0 runs a SINGLE 6-accumulator pass (q0..q3 + k + v): two
                # extra accumulators borrowed from the idle score pool (no
                # attention overlaps sb0's projections). 6 MMs per k-tile
                # halves the startup DMA demand per PE-cycle, which is what
                # the two HWDGE queues can actually sustain while weights and
                # x stream in cold.
                na = QH if sb == 0 else 2
                qa_ps = [ps_acc.tile([128, SB], f32, tag="acc", name="acc") for _ in range(2)]
                if sb == 0:
                    qa_ps += [ps_sc.tile([128, SB], f32, tag="sc_ps", name="sc_ps")
                              for _ in range(2)]
                k_ps = ps_acc.tile([128, SB], f32, tag="acc", name="acc")
                v_ps = ps_acc.tile([128, SB], f32, tag="acc", name="acc")
                xt_chunks = [None] * (KT // 4)
                for c4 in range(KT // 4):
                    if sb == 0:
                        if c4 == 0:
                            xt4 = xt_first
                        else:
                            # sb0's x all on sync: scalar stays a pure wq
                            # stream so neither starves the cold ramp
                            xt4 = xpool.tile([128, 4, SB], bft, tag="xt", name="xt")
                            nc.sync.dma_start(out=xt4[:], in_=xt_p[sb, c4])
                        load_wq_chunk(c4 + 2)
                        for lo, hi in kv_pieces.get(c4, []):
                            load_kv_piece(lo, hi, nc.sync)
                    else:
                        # pre-emitted at the previous sb's pass-B start
                        xt4 = xt_next[c4]
                    xt_chunks[c4] = xt4
                    for k4 in range(4):
                        kt = c4 * 4 + k4
                        xt = xt4[:, k4, :]
                        st = (kt == 0)
                        sp = (kt == KT - 1)
                        for h in range(na):
                            nc.tensor.matmul(
                                qa_ps[h][:], wq_at(kt, h * 128, (h + 1) * 128), xt,
                                start=st, stop=sp,
                            )
                        nc.tensor.matmul(k_ps[:], wk_sb[:, kt, :], xt, start=st, stop=sp)
                        nc.tensor.matmul(v_ps[:], wv_sb[:, kt, :], xt, start=st, stop=sp)

                if sb == 0:
                    # sb0's chunks are fully read now: pre-emit sb1's x DMAs
                    # (slots recycle progressively), then wo on scalar (which
                    # is idle after the wq stream; needed only at ~300us)
                    for c4 in range(KT // 4):
                        t = xpool.tile([128, 4, SB], bft, tag="xt", name="xt")
                        eng = nc.sync if c4 % 2 == 0 else nc.scalar
                        eng.dma_start(out=t[:], in_=xt_p[1, c4])
                        xt_next[c4] = t
                    for j in range(4):
                        nc.scalar.dma_start(out=wo_ch[j][:], in_=wo4[j])

                for h in range(na):
                    qraw = qpool.tile([128, SB], bft, tag="qraw", name="qraw")
                    nc.scalar.activation(qraw[:], qa_ps[h][:], AF.Identity,
                                         bias=bqc_sb[:, h:h + 1])
                    qr = qpool.tile([128, SB], bft, tag="qrope", name="qrope", bufs=8)
                    rope(qr, qraw, tkc_sb, tks_sb, s0)
                    q_sb[h] = qr
                kraw = qpool.tile([128, SB], bft, tag="kraw", name="kraw")
                nc.scalar.activation(kraw[:], k_ps[:], AF.Identity,
                                     bias=bkc_sb[:, 0:1])
                k_t = persist.tile([128, SB], bft, tag="k_t", name="k_t")
                rope(k_t, kraw, tkc_sb, tks_sb, s0)
                k_tiles[sb] = k_t
                vraw = qpool.tile([128, SB], bft, tag="vraw", name="vraw")
                nc.scalar.activation(vraw[:], v_ps[:], AF.Identity,
                                     bias=bvc_sb[:, 0:1])
                # transpose V on the PE (vraw_tile.T @ I) -- DMA_TRANSPOSE
                # serializes behind in-flight collectives, this doesn't
                vt_ps = ps_sc.tile([128, QH, 128], f32, tag="sc_ps", name="sc_ps")
                for i in range(QH):
                    nc.tensor.matmul(vt_ps[:, i, :], vraw[:, i * 128:(i + 1) * 128],
                                     ident_sb[:], start=True, stop=True)
                v_t = persist.tile([128, QH, 128], bft, tag="v_t", name="v_t")
                nc.vector.tensor_copy(v_t[:], vt_ps[:])
                v_tiles[sb] = v_t

                # ---- projection pass B (sb>0): q2, q3, x chunks resident.
                # Forward c4 order so chunk slots free progressively.
                if sb > 0:
                    qb_ps = [ps_acc.tile([128, SB], f32, tag="acc", name="acc") for _ in range(2)]
                    first_b = True
                    for c4 in range(KT // 4):
                        xt4 = xt_chunks[c4]
                        for k4 in range(4):
                            kt = c4 * 4 + k4
                            xt = xt4[:, k4, :]
                            last_b = (c4 == KT // 4 - 1 and k4 == 3)
                            for h in range(2):
                                nc.tensor.matmul(
                                    qb_ps[h][:], wq_at(kt, (2 + h) * 128, (3 + h) * 128), xt,
                                    start=first_b, stop=last_b,
                                )
                            first_b = False
                    # pre-emit the NEXT sb's x DMAs (after pass B's reads so
                    # the WAR slot reuse is tracked; the DMAs still fire
                    # during pass B execution, ahead of this sb's ctx DMAs)
                    if sb + 1 < NSB:
                        for c4 in range(KT // 4):
                            t = xpool.tile([128, 4, SB], bft, tag="xt", name="xt")
                            eng = nc.sync if c4 % 2 == 0 else nc.scalar
                            eng.dma_start(out=t[:], in_=xt_p[sb + 1, c4])
                            xt_next[c4] = t
                    for h in range(2):
                        qraw = qpool.tile([128, SB], bft, tag="qraw", name="qraw")
                        nc.scalar.activation(qraw[:], qb_ps[h][:], AF.Identity,
                                             bias=bqc_sb[:, 2 + h:3 + h])
                        qr = qpool.tile([128, SB], bft, tag="qrope", name="qrope", bufs=8)
                        rope(qr, qraw, tkc_sb, tks_sb, s0)
                        q_sb[2 + h] = qr

                # ---- causal attention for q-block sb, 4 heads ----
                nkt2 = 4 * (sb + 1)
                for h in range(QH):
                    ctx_ps = ps_cx.tile([128, SB], f32, tag="ctx_ps", name="ctx_ps")

                    ds = dsum.tile([128, SB], f32, tag="ds", name="ds")

                    def emit_score(kt2):
                        ksb, ki = divmod(kt2, 4)
                        off = 128 * ki if ksb == sb else 0
                        sc_ps = ps_sc.tile([128, SB], f32, tag="sc_ps", name="sc_ps")
                        nc.tensor.matmul(
                            sc_ps[:, off:],
                            k_tiles[ksb][:, ki * 128:(ki + 1) * 128],
                            q_sb[h][:, off:],
                            start=True, stop=True,
                        )
                        probs = ppool.tile([128, SB], bft, tag="probs", name="probs", bufs=7)
                        nc.scalar.activation(probs[:, off:], sc_ps[:, off:], AF.Exp,
                                             scale=float(SCALE))
                        if ksb == sb:
                            psel = ppool.tile([128, SB], bft, tag="psel", name="psel", bufs=3)
                            nc.vector.tensor_tensor(
                                psel[:, off:], probs[:, off:], cm_sb[:, ki, off:], ALU.mult
                            )
                            probs = psel
                        return probs, off

                    def accum_ds(probs, off, first):
                        # ds accumulates the (masked) probs at SCORE time --
                        # three tiles ahead of PV -- so the den/recip/bcast
                        # chain below overlaps the tail PV matmuls and only
                        # mult+DMA remain after the last PV lands.
                        if first:
                            nc.vector.tensor_copy(ds[:], probs[:])
                        else:
                            nc.vector.tensor_tensor(ds[:, off:], ds[:, off:],
                                                    probs[:, off:], ALU.add)

                    def emit_pv(kt2, probs, off):
                        ksb, ki = divmod(kt2, 4)
                        st = (kt2 == 0)
                        sp = (kt2 == nkt2 - 1)
                        nc.tensor.matmul(
                            ctx_ps[:, off:], v_tiles[ksb][:, ki, :], probs[:, off:],
                            start=st, stop=sp,
                        )

                    def emit_den_chain():
                        # gpsimd partition_all_reduce: den summed across
                        # partitions AND broadcast to all of them in one
                        # SBUF->SBUF op (~3.5us) -- no PE matmuls, no PSUM
                        # slots, no DRAM round trip. Then one DVE reciprocal.
                        # Emitted right after the last score so the result is
                        # ready when the last PV retires.
                        den_bc = npool.tile([128, SB], f32, tag="den_bc", name="den_bc")
                        nc.gpsimd.partition_all_reduce(den_bc[:], ds[:], 128,
                                                       bass_isa.ReduceOp.add)
                        rb_bc = npool.tile([128, SB], f32, tag="rb_bc", name="rb_bc")
                        nc.vector.reciprocal_approx_fast(rb_bc[:], den_bc[:])
                        return rb_bc

                    fifo = []
                    for k in range(min(3, nkt2)):
                        pr, off = emit_score(k)
                        accum_ds(pr, off, k == 0)
                        fifo.append((pr, off))
                    bc_s = None
                    for kt2 in range(nkt2):
                        nxt = kt2 + 3
                        if nxt < nkt2:
                            pr, off = emit_score(nxt)
                            accum_ds(pr, off, False)
                            fifo.append((pr, off))
                        if bc_s is None and nxt >= nkt2 - 1:
                            bc_s = emit_den_chain()
                        pr, off = fifo.pop(0)
                        emit_pv(kt2, pr, off)
                    ctx_sb = cpool.tile([128, SB], bft, tag="ctx_sb", name="ctx_sb")
                    nc.vector.tensor_tensor(ctx_sb[:], ctx_ps[:], bc_s[:], ALU.mult)
                    # HWDGE queues (gpsimd's SWDGE crawls at ~15-25GB/s, which
                    # delayed the AllGather start by ~40us per sb)
                    ceng = nc.sync if h % 2 == 0 else nc.scalar
                    ceng.dma_start(
                        out=cc_in[sb][h * 128:(h + 1) * 128, :], in_=ctx_sb[:]
                    )
                    # one AllGather per sb once all 4 heads' ctx is in DRAM
                    if h == QH - 1:
                        nc.gpsimd.collective_compute(
                            "AllGather",
                            ALU.bypass,
                            replica_groups=[list(range(N_CORES))],
                            ins=[cc_in[sb][:].opt()],
                            outs=[cc_out[sb][:].opt()],
                        )

            # all o_proj PE work stacked at the end: sb3's attention finishes
            # ~135us earlier, its AllGather overlaps o_proj(0..2), and the
            # scheduler fills attention-chain bubbles with o_proj matmuls
            for sb in range(NSB):
                o_proj(sb)

    nc.finalize()
    return nc


def _get_nc():
    if "nc" not in _CACHE:
        _CACHE["nc"] = _build_nc()
    return _CACHE["nc"]


def _make_in_maps(x, freqs_cos, freqs_sin, wq, bq, wk, bk, wv, bv, wo):
    x2 = np.ascontiguousarray(np.asarray(x).reshape(S, DIM))
    xT = np.ascontiguousarray(x2.T)
    # [NSB, KT//4, 128, 4, SB]: xt_p[sb, c4, p, k4, s'] = xT[128*(4c4+k4)+p, 512sb+s']
    xt_p = np.ascontiguousarray(
        xT.reshape(KT // 4, 4, 128, NSB, SB).transpose(3, 0, 2, 1, 4))
    cos = np.asarray(freqs_cos, dtype=np.float32)
    sin = np.asarray(freqs_sin, dtype=np.float32)
    def dup(t):
        return np.ascontiguousarray(np.concatenate([t, t], axis=0).astype(bf16))
    tkc_np = dup(cos.T)
    tks_np = dup(sin.T)
    jj = np.arange(SB)[None, None, :]
    pp = np.arange(128)[None, :, None]
    off = (np.arange(4) * 128)[:, None, None]
    cmask_np = np.ascontiguousarray((jj - off - pp >= 0).astype(bf16))
    wq = np.asarray(wq); wk = np.asarray(wk); wv = np.asarray(wv); wo = np.asarray(wo)
    bq = np.asarray(bq); bk = np.asarray(bk); bv = np.asarray(bv)
    in_maps = []
    for c in range(N_CORES):
        qs = slice(c * OC, (c + 1) * OC)
        ks = slice(c * HD, (c + 1) * HD)
        wqT_c = wq[qs].T.astype(bf16)   # [DIM, OC]
        wkT_c = wk[ks].T.astype(bf16)   # [DIM, HD]
        wvT_c = wv[ks].T.astype(bf16)
        woT_c = wo[qs].T.astype(bf16)

        def tile_w4(wT):
            # [DIM, O] -> [4, 128, KT//4, O]
            return np.ascontiguousarray(
                wT.reshape(4, KT // 4, 128, wT.shape[1]).transpose(0, 2, 1, 3))

        def tile_w8(wT):
            # [DIM, O] -> [8, 128, KT//8, O]
            return np.ascontiguousarray(
                wT.reshape(8, KT // 8, 128, wT.shape[1]).transpose(0, 2, 1, 3))

        def tile_wkv(wT):
            # [DIM, HD] -> [128, KT, HD]
            return np.ascontiguousarray(
                wT.reshape(KT, 128, wT.shape[1]).transpose(1, 0, 2))

        in_maps.append({
            "xt_p": xt_p,
            "wq8": tile_w8(wqT_c),
            "wk_p": tile_wkv(wkT_c),
            "wv_p": tile_wkv(wvT_c),
            "wo4": tile_w4(woT_c),
            "bqc": np.ascontiguousarray(bq[qs].astype(np.float32).reshape(QH, HD).T),
            "bkc": np.ascontiguousarray(bk[ks].astype(np.float32).reshape(1, HD).T),
            "bvc": np.ascontiguousarray(bv[ks].astype(np.float32).reshape(1, HD).T),
            "tkc": tkc_np,
            "tks": tks_np,
            "cmask": cmask_np,
            "ident": np.ascontiguousarray(np.eye(128, dtype=bf16)),
        })
    return in_maps


def _assemble(results):
    out = np.empty((S, DIM), dtype=bf16)
    for c, r in enumerate(results):
        out[:, c * OC:(c + 1) * OC] = np.asarray(r["outT"]).T
    return out.reshape(B, S, DIM)


def _mask_is_causal(mask):
    m = np.asarray(mask, dtype=np.float32)
    ii = np.arange(S, dtype=np.int64)
    expect = np.where(ii[None, :] <= ii[:, None], np.float32(0.0), np.float32(NEG))
    return m.shape == (S, S) and bool(np.array_equal(m, expect))


def _numpy_fallback(x, freqs_cos, freqs_sin, mask, wq, bq, wk, bk, wv, bv, wo):
    # exact replica of the reference in numpy (used only if mask isn't causal)
    xf = np.asarray(x).astype(np.float32).reshape(S, DIM)
    cos = np.asarray(freqs_cos, dtype=np.float32)
    sin = np.asarray(freqs_sin, dtype=np.float32)

    def tb(t):
        return np.asarray(t).astype(np.float32)

    xq = (xf @ tb(wq).T + tb(bq)).astype(bf16).astype(np.float32).reshape(S, H, HD)
    xk = (xf @ tb(wk).T + tb(bk)).astype(bf16).astype(np.float32).reshape(S, HKV, HD)
    xv = (xf @ tb(wv).T + tb(bv)).astype(bf16).astype(np.float32).reshape(S, HKV, HD)

    def rope_np(t):
        half = HD // 2
        a, b = t[..., :half], t[..., half:]
        c = cos[:, None, :]
        s = sin[:, None, :]
        return np.concatenate([a * c - b * s, a * s + b * c], axis=-1)

    xq = rope_np(xq).astype(bf16).astype(np.float32)
    xk = rope_np(xk).astype(bf16).astype(np.float32)
    key = np.repeat(xk, H // HKV, axis=1)
    val = np.repeat(xv, H // HKV, axis=1)
    scores = np.einsum("qhd,khd->hqk", xq, key) * SCALE
    scores = scores + np.asarray(mask, dtype=np.float32)[None]
    scores -= scores.max(axis=-1, keepdims=True)
    p = np.exp(scores)
    p /= p.sum(axis=-1, keepdims=True)
    ctx = np.einsum("hqk,khd->qhd", p.astype(bf16).astype(np.float32), val)
    ctx = ctx.reshape(S, H * HD).astype(bf16).astype(np.float32)
    out = (ctx @ tb(wo).T).astype(bf16)
    return out.reshape(B, S, DIM)


def kernel(x, freqs_cos, freqs_sin, mask, positions, wq, bq, wk, bk, wv, bv, wo,
           _trace=False, _tmpdir=None):
    from concourse.bass_utils import run_bass_kernel_spmd

    if not _mask_is_causal(mask):
        return _numpy_fallback(x, freqs_cos, freqs_sin, mask, wq, bq, wk, bk, wv, bv, wo)

    in_maps = _make_in_maps(x, freqs_cos, freqs_sin, wq, bq, wk, bk, wv, bv, wo)
    nc = _get_nc()
    res = run_bass_kernel_spmd(
        nc, in_maps, core_ids=list(range(N_CORES)), trace=_trace, tmpdir=_tmpdir
    )
    out = _assemble(res.results)
    if _trace:
        return out, res
    return out



# revision 49
# speedup vs baseline: 1.0862x; 1.0065x over previous
"""Trainium2 Bass kernel: GQA attention block (B=1, S=2048, DIM=4096, 32 Q / 8 KV
heads, HD=128, RoPE, causal mask, o_proj), tensor-parallel over 8 NeuronCores.

Sharding (per core c):
  - Q heads 4c..4c+3 (wq rows 512c..512c+512), KV head c (wk/wv rows 128c..).
  - x replicated; each core computes qkv projections + RoPE + causal attention
    for its heads, producing ctx^T [512 local features, 2048 seq] in bf16.
  - AllGather over the feature axis -> ctx^T full [4096, 2048], then each core
    computes o_proj for its 512 output columns (wo rows 512c..512c+512).
  - Host concatenates the per-core output column blocks.

Schedule notes (trace-driven):
  - PE is the bottleneck (~1630 512-free matmuls) and the board GPIO power
    throttle caps the sustained clock at 13/16 x 2.4GHz; idle gaps >3.4us
    additionally re-throttle to half (HAM). Goal: a gapless PE stream.
  - Phase order: [proj_k + attn_k for k=0..3] back-to-back, then ALL four
    o_proj phases at the end. Every AllGather (one 512KB op per sb, ~20-35us,
    serial on one CC stream) lands with ~100us of slack before its consumer,
    which also absorbs the 30-180us run-to-run CC stream-init stagger.
  - gpsimd carries ONLY the collective triggers (+ startup consts): anything
    else head-of-line-blocks the AG trigger behind data waits. ctx/rope-swap
    DMAs ride the two HWDGE queues; the next sb's x DMAs are pre-emitted at
    pass-B so they sit ahead of attention's DMAs in FIFO order.
  - sb0 projections run one 6-accumulator pass (q0..q3+k+v, 2 PSUM banks
    borrowed from the idle score pool) to halve startup DMA demand per cycle.
  - Softmax normalize stays on-chip: bf16 denominator matmul, DVE reciprocal,
    rank-1 PE matmul broadcasts 1/den across partitions (no DRAM round trip);
    softmax SCALE is folded into the Exp activation so q/k share rope tables.
  - Diagonal score/PV matmuls shrink their free (query) range causally.
"""

import numpy as np
import ml_dtypes

B, S, DIM = 1, 2048, 4096
H, HKV, HD = 32, 8, 128
N_CORES = 8
QH = H // N_CORES            # 4 local q heads
OC = QH * HD                 # 512 local q/out columns
SB = 512                     # seq block
NSB = S // SB                # 4
KT = DIM // 128              # 32 contraction tiles
SCALE = HD ** -0.5
NEG = -1e9

bf16 = ml_dtypes.bfloat16

_CACHE = {}


def _build_nc():
    import contextlib
    import concourse.tile as tile
    from concourse import bacc, bass_isa, library_config, mybir

    f32 = mybir.dt.float32
    bft = mybir.dt.bfloat16
    AF = mybir.ActivationFunctionType
    ALU = mybir.AluOpType

    nc = bacc.Bacc("TRN2")

    # pre-tiled on host: xt4[sb][c4] -> [128, 4, SB] contiguous; wq8[j] ->
    # [128, 4, OC] contiguous (k-tiles 4j..4j+3); wkv -> [128, KT, HD]
    xt_p = nc.declare_dram_parameter("xt_p", [NSB, KT // 4, 128, 4, SB], bft, isOutput=False)
    wq8 = nc.declare_dram_parameter("wq8", [8, 128, KT // 8, OC], bft, isOutput=False)
    wk_p = nc.declare_dram_parameter("wk_p", [128, KT, HD], bft, isOutput=False)
    wv_p = nc.declare_dram_parameter("wv_p", [128, KT, HD], bft, isOutput=False)
    wo4 = nc.declare_dram_parameter("wo4", [4, 128, KT // 4, OC], bft, isOutput=False)
    bqc = nc.declare_dram_parameter("bqc", [128, QH], mybir.dt.float32, isOutput=False)
    bkc = nc.declare_dram_parameter("bkc", [128, 1], mybir.dt.float32, isOutput=False)
    bvc = nc.declare_dram_parameter("bvc", [128, 1], mybir.dt.float32, isOutput=False)
    tkc = nc.declare_dram_parameter("tkc", [128, S], bft, isOutput=False)
    tks = nc.declare_dram_parameter("tks", [128, S], bft, isOutput=False)
    cmask = nc.declare_dram_parameter("cmask", [4, 128, SB], bft, isOutput=False)
    ident = nc.declare_dram_parameter("ident", [128, 128], bft, isOutput=False)
    outT = nc.declare_dram_parameter("outT", [OC, S], bft, isOutput=True)

    cc_warm_in = nc.dram_tensor("cc_warm_in", [1, 128], mybir.dt.float32)
    cc_warm_out = nc.dram_tensor("cc_warm_out", [N_CORES, 128], mybir.dt.float32,
                                 addr_space="Shared")
    cc_in = [nc.dram_tensor(f"cc_in{sb}", [OC, SB], bft) for sb in range(NSB)]
    # one AllGather per sb: all o_proj PE work is stacked at the END of the
    # kernel (after sb3's attention), so every gather lands with ~100us of
    # slack before its consumer -- fewer ops under the ~15-20us/op fixed cost,
    # and robust to the run-to-run CC stream-init stagger.
    cc_out = [nc.dram_tensor(f"cc_out{sb}", [N_CORES * OC, SB], bft,
                             addr_space="Shared") for sb in range(NSB)]

    with tile.TileContext(nc) as tc:
        with contextlib.ExitStack() as ctx:
            consts = ctx.enter_context(tc.tile_pool(name="consts", bufs=1))
            xpool = ctx.enter_context(tc.tile_pool(name="xpool", bufs=10))
            persist = ctx.enter_context(tc.tile_pool(name="persist", bufs=4))
            qpool = ctx.enter_context(tc.tile_pool(name="qpool", bufs=2))
            rtmp = ctx.enter_context(tc.tile_pool(name="rtmp", bufs=2))
            ppool = ctx.enter_context(tc.tile_pool(name="ppool", bufs=6))
            npool = ctx.enter_context(tc.tile_pool(name="npool", bufs=2))
            dsum = ctx.enter_context(tc.tile_pool(name="dsum", bufs=2))
            cpool = ctx.enter_context(tc.tile_pool(name="cpool", bufs=3))
            opool = ctx.enter_context(tc.tile_pool(name="opool", bufs=3))

            ps_acc = ctx.enter_context(tc.tile_pool(name="ps_acc", bufs=4, space="PSUM"))
            ps_sc = ctx.enter_context(tc.tile_pool(name="ps_sc", bufs=3, space="PSUM"))
            ps_cx = ctx.enter_context(tc.tile_pool(name="ps_cx", bufs=1, space="PSUM"))

            # tiny dummy collective, FIRST gpsimd instruction: queues behind
            # the runtime's CC init barrier so the first real AllGather starts
            # without the ~45us stream-init latency. The gathered bytes are
            # garbage and never read.
            nc.gpsimd.collective_compute(
                "AllGather", ALU.bypass,
                replica_groups=[list(range(N_CORES))],
                ins=[cc_warm_in[:].opt()],
                outs=[cc_warm_out[:].opt()],
            )
            # partition_all_reduce (softmax denominator) lives in the attn
            # gpsimd library
            nc.gpsimd.load_library(library_config.attn)

            # ---- priority DMAs: exactly what the first projection MMs need,
            # spread over FOUR trigger queues so no single queue serializes the
            # startup: sync=x stream, scalar=wq chunks, vector=wk/wv pieces
            # (+ident, then wo), gpsimd=rope tables+mask.
            KC = KT // 4
            KQ = KT // 8
            wq_ch = [None] * 8

            def load_wq_chunk(j, eng=None):
                if j < 8 and wq_ch[j] is None:
                    w = consts.tile([128, KQ, OC], bft, tag=f"wq{j}", name=f"wq{j}")
                    (eng or nc.scalar).dma_start(out=w[:], in_=wq8[j])
                    wq_ch[j] = w

            # memset for PE warmup first on the vector FIFO (no deps, fast)
            wtile = consts.tile([128, SB], bft)
            nc.vector.memset(wtile[:], 0.0)

            xt_first = xpool.tile([128, 4, SB], bft, tag="xt", name="xt")
            nc.sync.dma_start(out=xt_first[:, 0:2, :], in_=xt_p[0, 0][:, 0:2, :])
            nc.sync.dma_start(out=xt_first[:, 2:4, :], in_=xt_p[0, 0][:, 2:4, :])
            wk_sb = consts.tile([128, KT, HD], bft)
            wv_sb = consts.tile([128, KT, HD], bft)

            def load_kv_piece(lo, hi, eng):
                eng.dma_start(out=wk_sb[:, lo:hi, :], in_=wk_p[:, lo:hi, :])
                eng.dma_start(out=wv_sb[:, lo:hi, :], in_=wv_p[:, lo:hi, :])

            # scalar = pure wq stream (4MB, chunk j lands well before its
            # k-tiles are reached); sync = x + wk/wv interleaved in
            # consumption order. ~5MB each by pass-A end.
            load_wq_chunk(0)
            load_wq_chunk(1)
            bqc_sb = consts.tile([128, QH], f32)
            nc.sync.dma_start(out=bqc_sb[:], in_=bqc[:])
            bkc_sb = consts.tile([128, 1], f32)
            nc.sync.dma_start(out=bkc_sb[:], in_=bkc[:])
            bvc_sb = consts.tile([128, 1], f32)
            nc.sync.dma_start(out=bvc_sb[:], in_=bvc[:])
            ident_sb = consts.tile([128, 128], bft)
            nc.sync.dma_start(out=ident_sb[:], in_=ident[:])
            load_kv_piece(0, 4, nc.sync)
            load_kv_piece(4, 8, nc.sync)
            # remaining wk/wv pieces interleave with sb0's x stream on sync
            # (emitted inside the c4 loop, keyed by consumption time)
            kv_pieces = {c4: [(4 * c4 + 4, 4 * c4 + 8)] for c4 in range(1, 7)}

            # PE warmup: keep the clock up while the first DMAs land. Short --
            # the first projection matmuls should be ready right after.
            warm_ps = ps_sc.tile([128, SB], f32, tag="sc_ps", name="sc_ps")
            for i in range(18):
                nc.tensor.matmul(warm_ps[:], wtile[:, 0:128], wtile[:],
                                 start=(i == 0), stop=(i == 17))

            # bulk consts on gpsimd: rope tables + mask (needed ~45us). One
            # cos/sin table pair serves q AND k -- the softmax SCALE is folded
            # into the Exp activation's scale operand instead of q's tables.
            tkc_sb = consts.tile([128, S], bft)
            nc.gpsimd.dma_start(out=tkc_sb[:], in_=tkc[:])
            tks_sb = consts.tile([128, S], bft)
            nc.gpsimd.dma_start(out=tks_sb[:], in_=tks[:])
            cm_sb = consts.tile([128, 4, SB], bft)
            nc.gpsimd.dma_start(out=cm_sb[:], in_=cmask.rearrange("j p q -> p j q"))
            # o_proj weight tiles: DMAs emitted on scalar at sb0 pass B (behind
            # the projection weights, ahead of sb1's x stream; needed ~150us)
            wo_ch = [consts.tile([128, KC, OC], bft, tag=f"wo{j}", name=f"wo{j}")
                     for j in range(4)]


            def wq_at(kt, lo, hi):
                return wq_ch[kt // KQ][:, kt % KQ, lo:hi]

            def wo_at(kt, lo, hi):
                return wo_ch[kt // KC][:, kt % KC, lo:hi]

            # persistent per-sb K^T and V tiles
            k_tiles = [None] * NSB   # [128 d, SB s] bf16
            v_tiles = [None] * NSB   # [128 s, 4, 128 d] bf16

            rope_ctr = [0]

            def rope(dst, src, tcos, tsin, s0):
                # dst/src: [128, SB] bf16. tcos/tsin have cos/sin duplicated in
                # both partition halves. swp = src with halves swapped (DMA
                # partition move), so every DVE op is partition-aligned.
                # Swap DMAs ride the HWDGE queues (alternating): gpsimd must
                # stay empty so AllGather triggers fire the moment ctx lands
                # (the tile scheduler queues swp DMAs ahead of AG triggers,
                # and a swp head-of-line-blocks until its qraw exists).
                cL = tcos[0:64, s0:s0 + SB]
                cH = tcos[64:128, s0:s0 + SB]
                sL = tsin[0:64, s0:s0 + SB]
                sH = tsin[64:128, s0:s0 + SB]
                swp = rtmp.tile([128, SB], bft, tag="ropeswp", name="ropeswp")
                seng = nc.sync if rope_ctr[0] % 2 == 0 else nc.scalar
                rope_ctr[0] += 1
                seng.dma_start(out=swp[0:64, :], in_=src[64:128, :])
                seng.dma_start(out=swp[64:128, :], in_=src[0:64, :])
                tA = rtmp.tile([128, SB], bft, tag="ropetA", name="ropetA")
                tB = rtmp.tile([128, SB], bft, tag="ropetB", name="ropetB")
                nc.vector.tensor_tensor(tA[0:64, :], src[0:64, :], cL, ALU.mult)
                nc.vector.tensor_tensor(tA[64:128, :], swp[64:128, :], sH, ALU.mult)
                nc.vector.tensor_tensor(tB[0:64, :], swp[0:64, :], sL, ALU.mult)
                nc.vector.tensor_tensor(tB[64:128, :], src[64:128, :], cH, ALU.mult)
                nc.vector.tensor_tensor(dst[0:64, :], tA[0:64, :], tB[0:64, :], ALU.subtract)
                nc.vector.tensor_tensor(dst[64:128, :], tA[64:128, :], tB[64:128, :], ALU.add)

            def o_proj(sb):
                s0 = sb * SB
                o_ps = [ps_acc.tile([128, SB], f32, tag="acc", name="acc") for _ in range(QH)]
                # cc_out rows: core*512 + head*128 + p, so chunk feature-tile
                # b = c4*4+k4 maps 1:1 to the wo contraction tile index
                co_r = cc_out[sb].rearrange("(c b p) s -> c p b s", p=128, b=4)
                for c4 in range(8):
                    rt4 = opool.tile([128, 4, SB], bft, tag="rt", name="rt", bufs=4)
                    eng = nc.sync if c4 % 2 == 0 else nc.scalar
                    eng.dma_start(out=rt4[:], in_=co_r[c4])
                    for k4 in range(4):
                        ft = c4 * 4 + k4
                        for ct in range(QH):
                            nc.tensor.matmul(
                                o_ps[ct][:], wo_at(ft, ct * 128, (ct + 1) * 128),
                                rt4[:, k4, :],
                                start=(ft == 0), stop=(ft == KT - 1),
                            )
                for ct in range(QH):
                    ot = opool.tile([128, SB], bft, tag="ot", name="ot")
                    nc.vector.tensor_copy(ot[:], o_ps[ct][:])
                    nc.scalar.dma_start(
                        out=outT[ct * 128:(ct + 1) * 128, s0:s0 + SB], in_=ot[:]
                    )

            xt_next = [None] * (KT // 4)
            for sb in range(NSB):
                s0 = sb * SB
                q_sb = [None] * QH

                # ---- projection pass A ----
                # sb0 runs a SINGLE 6-accumulator pass (q0..q3 + k + v): two
                # extra accumulators borrowed from the idle score pool (no
                # attention overlaps sb0's projections). 6 MMs per k-tile
                # halves the startup DMA demand per PE-cycle, which is what
                # the two HWDGE queues can actually sustain while weights and
                # x stream in cold.
                na = QH if sb == 0 else 2
                qa_ps = [ps_acc.tile([128, SB], f32, tag="acc", name="acc") for _ in range(2)]
                if sb == 0:
                    qa_ps += [ps_sc.tile([128, SB], f32, tag="sc_ps", name="sc_ps")
                              for _ in range(2)]
                k_ps = ps_acc.tile([128, SB], f32, tag="acc", name="acc")
                v_ps = ps_acc.tile([128, SB], f32, tag="acc", name="acc")
                xt_chunks = [None] * (KT // 4)
                for c4 in range(KT // 4):
                    if sb == 0:
                        if c4 == 0:
                            xt4 = xt_first
                        else:
                            # sb0's x all on sync: scalar stays a pure wq
                            # stream so neither starves the cold ramp
                            xt4 = xpool.tile([128, 4, SB], bft, tag="xt", name="xt")
                            nc.sync.dma_start(out=xt4[:], in_=xt_p[sb, c4])
                        load_wq_chunk(c4 + 2)
                        for lo, hi in kv_pieces.get(c4, []):
                            load_kv_piece(lo, hi, nc.sync)
                    else:
                        # pre-emitted at the previous sb's pass-B start
                        xt4 = xt_next[c4]
                    xt_chunks[c4] = xt4
                    for k4 in range(4):
                        kt = c4 * 4 + k4
                        xt = xt4[:, k4, :]
                        st = (kt == 0)
                        sp = (kt == KT - 1)
                        for h in range(na):
                            nc.tensor.matmul(
                                qa_ps[h][:], wq_at(kt, h * 128, (h + 1) * 128), xt,
                                start=st, stop=sp,
                            )
                        nc.tensor.matmul(k_ps[:], wk_sb[:, kt, :], xt, start=st, stop=sp)
                        nc.tensor.matmul(v_ps[:], wv_sb[:, kt, :], xt, start=st, stop=sp)

                if sb == 0:
                    # sb0's chunks are fully read now: pre-emit sb1's x DMAs
                    # (slots recycle progressively), then wo on scalar (which
                    # is idle after the wq stream; needed only at ~300us)
                    for c4 in range(KT // 4):
                        t = xpool.tile([128, 4, SB], bft, tag="xt", name="xt")
                        eng = nc.sync if c4 % 2 == 0 else nc.scalar
                        eng.dma_start(out=t[:], in_=xt_p[1, c4])
                        xt_next[c4] = t
                    for j in range(4):
                        nc.scalar.dma_start(out=wo_ch[j][:], in_=wo4[j])

                for h in range(na):
                    qraw = qpool.tile([128, SB], bft, tag="qraw", name="qraw")
                    nc.vector.tensor_scalar_add(qraw[:], qa_ps[h][:],
                                                bqc_sb[:, h:h + 1])
                    qr = qpool.tile([128, SB], bft, tag="qrope", name="qrope", bufs=8)
                    rope(qr, qraw, tkc_sb, tks_sb, s0)
                    q_sb[h] = qr
                kraw = qpool.tile([128, SB], bft, tag="kraw", name="kraw")
                nc.vector.tensor_scalar_add(kraw[:], k_ps[:], bkc_sb[:, 0:1])
                k_t = persist.tile([128, SB], bft, tag="k_t", name="k_t")
                rope(k_t, kraw, tkc_sb, tks_sb, s0)
                k_tiles[sb] = k_t
                vraw = qpool.tile([128, SB], bft, tag="vraw", name="vraw")
                nc.vector.tensor_scalar_add(vraw[:], v_ps[:], bvc_sb[:, 0:1])
                # transpose V on the PE (vraw_tile.T @ I) -- DMA_TRANSPOSE
                # serializes behind in-flight collectives, this doesn't
                vt_ps = ps_sc.tile([128, QH, 128], f32, tag="sc_ps", name="sc_ps")
                for i in range(QH):
                    nc.tensor.matmul(vt_ps[:, i, :], vraw[:, i * 128:(i + 1) * 128],
                                     ident_sb[:], start=True, stop=True)
                v_t = persist.tile([128, QH, 128], bft, tag="v_t", name="v_t")
                nc.vector.tensor_copy(v_t[:], vt_ps[:])
                v_tiles[sb] = v_t

                # ---- projection pass B (sb>0): q2, q3, x chunks resident.
                # Forward c4 order so chunk slots free progressively.
                if sb > 0:
                    qb_ps = [ps_acc.tile([128, SB], f32, tag="acc", name="acc") for _ in range(2)]
                    first_b = True
                    for c4 in range(KT // 4):
                        xt4 = xt_chunks[c4]
                        for k4 in range(4):
                            kt = c4 * 4 + k4
                            xt = xt4[:, k4, :]
                            last_b = (c4 == KT // 4 - 1 and k4 == 3)
                            for h in range(2):
                                nc.tensor.matmul(
                                    qb_ps[h][:], wq_at(kt, (2 + h) * 128, (3 + h) * 128), xt,
                                    start=first_b, stop=last_b,
                                )
                            first_b = False
                    # pre-emit the NEXT sb's x DMAs (after pass B's reads so
                    # the WAR slot reuse is tracked; the DMAs still fire
                    # during pass B execution, ahead of this sb's ctx DMAs)
                    if sb + 1 < NSB:
                        for c4 in range(KT // 4):
                            t = xpool.tile([128, 4, SB], bft, tag="xt", name="xt")
                            eng = nc.sync if c4 % 2 == 0 else nc.scalar
                            eng.dma_start(out=t[:], in_=xt_p[sb + 1, c4])
                            xt_next[c4] = t
                    for h in range(2):
                        qraw = qpool.tile([128, SB], bft, tag="qraw", name="qraw")
                        nc.vector.tensor_scalar_add(qraw[:], qb_ps[h][:],
                                                    bqc_sb[:, 2 + h:3 + h])
                        qr = qpool.tile([128, SB], bft, tag="qrope", name="qrope", bufs=8)
                        rope(qr, qraw, tkc_sb, tks_sb, s0)
                        q_sb[2 + h] = qr

                # ---- causal attention for q-block sb, 4 heads ----
                nkt2 = 4 * (sb + 1)
                for h in range(QH):
                    ctx_ps = ps_cx.tile([128, SB], f32, tag="ctx_ps", name="ctx_ps")

                    ds = dsum.tile([128, SB], f32, tag="ds", name="ds")

                    def emit_score(kt2):
                        ksb, ki = divmod(kt2, 4)
                        off = 128 * ki if ksb == sb else 0
                        sc_ps = ps_sc.tile([128, SB], f32, tag="sc_ps", name="sc_ps")
                        nc.tensor.matmul(
                            sc_ps[:, off:],
                            k_tiles[ksb][:, ki * 128:(ki + 1) * 128],
                            q_sb[h][:, off:],
                            start=True, stop=True,
                        )
                        probs = ppool.tile([128, SB], bft, tag="probs", name="probs", bufs=7)
                        nc.scalar.activation(probs[:, off:], sc_ps[:, off:], AF.Exp,
                                             scale=float(SCALE))
                        if ksb == sb:
                            psel = ppool.tile([128, SB], bft, tag="psel", name="psel", bufs=3)
                            nc.vector.tensor_tensor(
                                psel[:, off:], probs[:, off:], cm_sb[:, ki, off:], ALU.mult
                            )
                            probs = psel
                        return probs, off

                    def accum_ds(probs, off, first):
                        # ds accumulates the (masked) probs at SCORE time --
                        # three tiles ahead of PV -- so the den/recip/bcast
                        # chain below overlaps the tail PV matmuls and only
                        # mult+DMA remain after the last PV lands.
                        if first:
                            nc.vector.tensor_copy(ds[:], probs[:])
                        else:
                            nc.vector.tensor_tensor(ds[:, off:], ds[:, off:],
                                                    probs[:, off:], ALU.add)

                    def emit_pv(kt2, probs, off):
                        ksb, ki = divmod(kt2, 4)
                        st = (kt2 == 0)
                        sp = (kt2 == nkt2 - 1)
                        nc.tensor.matmul(
                            ctx_ps[:, off:], v_tiles[ksb][:, ki, :], probs[:, off:],
                            start=st, stop=sp,
                        )

                    def emit_den_chain():
                        # gpsimd partition_all_reduce: den summed across
                        # partitions AND broadcast to all of them in one
                        # SBUF->SBUF op (~3.5us) -- no PE matmuls, no PSUM
                        # slots, no DRAM round trip. Then one DVE reciprocal.
                        # Emitted right after the last score so the result is
                        # ready when the last PV retires.
                        den_bc = npool.tile([128, SB], f32, tag="den_bc", name="den_bc")
                        nc.gpsimd.partition_all_reduce(den_bc[:], ds[:], 128,
                                                       bass_isa.ReduceOp.add)
                        rb_bc = npool.tile([128, SB], f32, tag="rb_bc", name="rb_bc")
                        nc.vector.reciprocal_approx_fast(rb_bc[:], den_bc[:])
                        return rb_bc

                    fifo = []
                    for k in range(min(3, nkt2)):
                        pr, off = emit_score(k)
                        accum_ds(pr, off, k == 0)
                        fifo.append((pr, off))
                    bc_s = None
                    for kt2 in range(nkt2):
                        nxt = kt2 + 3
                        if nxt < nkt2:
                            pr, off = emit_score(nxt)
                            accum_ds(pr, off, False)
                            fifo.append((pr, off))
                        if bc_s is None and nxt >= nkt2 - 1:
                            bc_s = emit_den_chain()
                        pr, off = fifo.pop(0)
                        emit_pv(kt2, pr, off)
                    ctx_sb = cpool.tile([128, SB], bft, tag="ctx_sb", name="ctx_sb")
                    nc.vector.tensor_tensor(ctx_sb[:], ctx_ps[:], bc_s[:], ALU.mult)
                    # HWDGE queues (gpsimd's SWDGE crawls at ~15-25GB/s, which
                    # delayed the AllGather start by ~40us per sb)
                    ceng = nc.sync if h % 2 == 0 else nc.scalar
                    ceng.dma_start(
                        out=cc_in[sb][h * 128:(h + 1) * 128, :], in_=ctx_sb[:]
                    )
                    # one AllGather per sb once all 4 heads' ctx is in DRAM
                    if h == QH - 1:
                        nc.gpsimd.collective_compute(
                            "AllGather",
                            ALU.bypass,
                            replica_groups=[list(range(N_CORES))],
                            ins=[cc_in[sb][:].opt()],
                            outs=[cc_out[sb][:].opt()],
                        )

            # all o_proj PE work stacked at the end: sb3's attention finishes
            # ~135us earlier, its AllGather overlaps o_proj(0..2), and the
            # scheduler fills attention-chain bubbles with o_proj matmuls
            for sb in range(NSB):
                o_proj(sb)

    nc.finalize()
    return nc


def _get_nc():
    if "nc" not in _CACHE:
        _CACHE["nc"] = _build_nc()
    return _CACHE["nc"]


def _make_in_maps(x, freqs_cos, freqs_sin, wq, bq, wk, bk, wv, bv, wo):
    x2 = np.ascontiguousarray(np.asarray(x).reshape(S, DIM))
    xT = np.ascontiguousarray(x2.T)
    # [NSB, KT//4, 128, 4, SB]: xt_p[sb, c4, p, k4, s'] = xT[128*(4c4+k4)+p, 512sb+s']
    xt_p = np.ascontiguousarray(
        xT.reshape(KT // 4, 4, 128, NSB, SB).transpose(3, 0, 2, 1, 4))
    cos = np.asarray(freqs_cos, dtype=np.float32)
    sin = np.asarray(freqs_sin, dtype=np.float32)
    def dup(t):
        return np.ascontiguousarray(np.concatenate([t, t], axis=0).astype(bf16))
    tkc_np = dup(cos.T)
    tks_np = dup(sin.T)
    jj = np.arange(SB)[None, None, :]
    pp = np.arange(128)[None, :, None]
    off = (np.arange(4) * 128)[:, None, None]
    cmask_np = np.ascontiguousarray((jj - off - pp >= 0).astype(bf16))
    wq = np.asarray(wq); wk = np.asarray(wk); wv = np.asarray(wv); wo = np.asarray(wo)
    bq = np.asarray(bq); bk = np.asarray(bk); bv = np.asarray(bv)
    in_maps = []
    for c in range(N_CORES):
        qs = slice(c * OC, (c + 1) * OC)
        ks = slice(c * HD, (c + 1) * HD)
        wqT_c = wq[qs].T.astype(bf16)   # [DIM, OC]
        wkT_c = wk[ks].T.astype(bf16)   # [DIM, HD]
        wvT_c = wv[ks].T.astype(bf16)
        woT_c = wo[qs].T.astype(bf16)

        def tile_w4(wT):
            # [DIM, O] -> [4, 128, KT//4, O]
            return np.ascontiguousarray(
                wT.reshape(4, KT // 4, 128, wT.shape[1]).transpose(0, 2, 1, 3))

        def tile_w8(wT):
            # [DIM, O] -> [8, 128, KT//8, O]
            return np.ascontiguousarray(
                wT.reshape(8, KT // 8, 128, wT.shape[1]).transpose(0, 2, 1, 3))

        def tile_wkv(wT):
            # [DIM, HD] -> [128, KT, HD]
            return np.ascontiguousarray(
                wT.reshape(KT, 128, wT.shape[1]).transpose(1, 0, 2))

        in_maps.append({
            "xt_p": xt_p,
            "wq8": tile_w8(wqT_c),
            "wk_p": tile_wkv(wkT_c),
            "wv_p": tile_wkv(wvT_c),
            "wo4": tile_w4(woT_c),
            "bqc": np.ascontiguousarray(bq[qs].astype(np.float32).reshape(QH, HD).T),
            "bkc": np.ascontiguousarray(bk[ks].astype(np.float32).reshape(1, HD).T),
            "bvc": np.ascontiguousarray(bv[ks].astype(np.float32).reshape(1, HD).T),
            "tkc": tkc_np,
            "tks": tks_np,
            "cmask": cmask_np,
            "ident": np.ascontiguousarray(np.eye(128, dtype=bf16)),
        })
    return in_maps


def _assemble(results):
    out = np.empty((S, DIM), dtype=bf16)
    for c, r in enumerate(results):
        out[:, c * OC:(c + 1) * OC] = np.asarray(r["outT"]).T
    return out.reshape(B, S, DIM)


def _mask_is_causal(mask):
    m = np.asarray(mask, dtype=np.float32)
    ii = np.arange(S, dtype=np.int64)
    expect = np.where(ii[None, :] <= ii[:, None], np.float32(0.0), np.float32(NEG))
    return m.shape == (S, S) and bool(np.array_equal(m, expect))


def _numpy_fallback(x, freqs_cos, freqs_sin, mask, wq, bq, wk, bk, wv, bv, wo):
    # exact replica of the reference in numpy (used only if mask isn't causal)
    xf = np.asarray(x).astype(np.float32).reshape(S, DIM)
    cos = np.asarray(freqs_cos, dtype=np.float32)
    sin = np.asarray(freqs_sin, dtype=np.float32)

    def tb(t):
        return np.asarray(t).astype(np.float32)

    xq = (xf @ tb(wq).T + tb(bq)).astype(bf16).astype(np.float32).reshape(S, H, HD)
    xk = (xf @ tb(wk).T + tb(bk)).astype(bf16).astype(np.float32).reshape(S, HKV, HD)
    xv = (xf @ tb(wv).T + tb(bv)).astype(bf16).astype(np.float32).reshape(S, HKV, HD)

    def rope_np(t):
        half = HD // 2
        a, b = t[..., :half], t[..., half:]
        c = cos[:, None, :]
        s = sin[:, None, :]
        return np.concatenate([a * c - b * s, a * s + b * c], axis=-1)

    xq = rope_np(xq).astype(bf16).astype(np.float32)
    xk = rope_np(xk).astype(bf16).astype(np.float32)
    key = np.repeat(xk, H // HKV, axis=1)
    val = np.repeat(xv, H // HKV, axis=1)
    scores = np.einsum("qhd,khd->hqk", xq, key) * SCALE
    scores = scores + np.asarray(mask, dtype=np.float32)[None]
    scores -= scores.max(axis=-1, keepdims=True)
    p = np.exp(scores)
    p /= p.sum(axis=-1, keepdims=True)
    ctx = np.einsum("hqk,khd->qhd", p.astype(bf16).astype(np.float32), val)
    ctx = ctx.reshape(S, H * HD).astype(bf16).astype(np.float32)
    out = (ctx @ tb(wo).T).astype(bf16)
    return out.reshape(B, S, DIM)


def kernel(x, freqs_cos, freqs_sin, mask, positions, wq, bq, wk, bk, wv, bv, wo,
           _trace=False, _tmpdir=None):
    from concourse.bass_utils import run_bass_kernel_spmd

    if not _mask_is_causal(mask):
        return _numpy_fallback(x, freqs_cos, freqs_sin, mask, wq, bq, wk, bk, wv, bv, wo)

    in_maps = _make_in_maps(x, freqs_cos, freqs_sin, wq, bq, wk, bk, wv, bv, wo)
    nc = _get_nc()
    res = run_bass_kernel_spmd(
        nc, in_maps, core_ids=list(range(N_CORES)), trace=_trace, tmpdir=_tmpdir
    )
    out = _assemble(res.results)
    if _trace:
        return out, res
    return out

